# revision 1
# baseline (speedup 1.0000x reference)
"""CrossTableAttention Trainium2 kernel (8-core SPMD, batch-sharded).

Math (per table t, row b):
  rel_w[t,r]   = sigmoid(rel_embs[t,r] . w_rel + b_rel)          (host, tiny)
  Qp[t]        = emb[t] @ Wq.T (+bq)                              [B, D]
  Kb[j]        = emb[j] @ Wk.T        (bias bk is softmax-invariant -> dropped)
  Vb[j]        = emb[j] @ Wv.T        (bias bv folded into output bias)
  score[t,b,h,r] = rel_w[t,r] * (Qp[t,b,h,:] . Kb[j_r,b,h,:]) / sqrt(DH)
  attn         = softmax_r(score);  P = attn * rel_w
  ctx[t,b]     = sum_r P[t,b,h,r] * Vb[j_r,b,h,:]
  out[t]       = ctx[t] @ Wo.T + (Wo @ bv + bo)

Key algebraic optimization: K/V projections are computed per *table* (16) instead
of per (table, relation) gather (128) - the relation weight is a scalar that
commutes with the linear projection.  5x fewer matmul FLOPs than the reference.

Device layout notes:
 - Activations live as [row(b) on partitions, feature on free] so the attention
   inner products reduce along the free axis (DVE).
 - q/k/v/ctx feature axis is permuted to dh-major (f = dh*16 + h) so the
   per-(b,h) attention-weight broadcast has a step-1 innermost AP dim
   (keeps DVE tensor_tensor in 2x bf16 mode).  Weight matrices are permuted
   host-side to produce/consume this layout directly.
 - scores/P layout per t: [128 b, 128 = r*16 + h] (h innermost).
 - Matmul operands need the contraction dim (d) on partitions; embT is produced
   by bf16 DMA-xbar transposing loads straight from the (host pre-cast) input.
"""

import sys

sys.path.insert(0, "/opt/trn_rl_repo")

import numpy as np
import ml_dtypes

import concourse.bass as bass
import concourse.bacc as bacc_mod
import concourse.mybir as mybir
import concourse.tile as tile
from concourse.bass_utils import run_bass_kernel_spmd

T, B, D, R, H = 16, 1024, 1024, 8, 16
DH = D // H  # 64
NCORES = 8
BC = B // NCORES  # 128 rows per core
KCH = D // 128  # 8 contraction chunks

F32 = mybir.dt.float32
BF16 = mybir.dt.bfloat16
AX = mybir.AxisListType
AF = mybir.ActivationFunctionType

# feature permutation: new f = dh*16 + h  <->  old o = h*64 + dh
_PERM = np.array([(f % H) * DH + f // H for f in range(D)], dtype=np.int64)


def _bcast_free(ap, n, pos):
    """Insert a [step=0, n] broadcast dim into an AP's free dims at `pos`
    (pos counts free dims, 0 = outermost free dim)."""
    new = list(ap.ap)
    new.insert(1 + pos, [0, n])
    return bass.AP(tensor=ap.tensor, offset=ap.offset, ap=new)


def _bcast_part(ap, n=128):
    """Broadcast a (DRAM) AP across n partitions by prepending a [0, n] dim."""
    return bass.AP(tensor=ap.tensor, offset=ap.offset, ap=[[0, n]] + list(ap.ap))


def _structure(rel_idx):
    """Host-side dedup of the (t, j) gather structure."""
    groups = []  # per t: list of (j, r0, [extra r's])
    for t in range(T):
        by_j = {}
        for r in range(R):
            by_j.setdefault(int(rel_idx[t, r]), []).append(r)
        groups.append([(j, rs[0], rs[1:]) for j, rs in sorted(by_j.items())])
    pairs_by_j = {j: [] for j in range(T)}
    for t in range(T):
        for j, r0, extras in groups[t]:
            pairs_by_j[j].append((t, r0))
    # first j (in ascending j processing order) touching each t
    first_j = {}
    for j in range(T):
        for t, _ in pairs_by_j[j]:
            if t not in first_j:
                first_j[t] = j
    # last j touching each t (to schedule ctx output DMA)
    last_j = {}
    for j in range(T):
        for t, _ in pairs_by_j[j]:
            last_j[t] = j
    return groups, pairs_by_j, first_j, last_j


def _build(rel_idx, use_bq, use_bo):
    """Build the SPMD bass program (identical on all cores).

    v2 structure (from v1 trace analysis):
     - Q+K projections interleaved per t (shared stationary embT chunk ->
       LDWEIGHTS amortized over 4 matmuls), Kb fully SBUF-resident so the
       scores phase never back-pressures the PE.
     - score pairs emitted in availability order (sorted by max(t, j)) so the
       DVE trails the projection stream without stalls.
     - per-head dot products via a binary tree of tensor_tensor adds (bf16 2x
       mode for the large levels, fp32 tail) instead of 1x tensor_reduce.
     - V projections recomputed in the ctx phase (keeps PE busy there);
       ctx accumulation adds alternate DVE / GPSIMD.
    """
    groups, pairs_by_j, first_j, last_j = _structure(rel_idx)

    # (t, j, r0, extras) sorted by when both Qp[t] and Kb[j] become available
    pair_sched = []
    for t in range(T):
        for j, r0, extras in groups[t]:
            pair_sched.append((max(t, j), t, j, r0, extras))
    pair_sched.sort(key=lambda x: (x[0], x[1]))
    last_pair_of_t = {}
    for i, (_, t, j, r0, ex) in enumerate(pair_sched):
        last_pair_of_t[t] = i

    nc = bacc_mod.Bacc(None, target_bir_lowering=False, debug=False)
    emb_ext = nc.dram_tensor("emb", [T, BC, D], BF16, kind="ExternalInput")
    wq_ext = nc.dram_tensor("wq", [D, D], BF16, kind="ExternalInput")
    wk_ext = nc.dram_tensor("wk", [D, D], BF16, kind="ExternalInput")
    wv_ext = nc.dram_tensor("wv", [D, D], BF16, kind="ExternalInput")
    wo_ext = nc.dram_tensor("wo", [D, D], BF16, kind="ExternalInput")
    rw_ext = nc.dram_tensor("rw", [T, R * H], F32, kind="ExternalInput")
    if use_bq:
        bq_ext = nc.dram_tensor("bqp", [D], F32, kind="ExternalInput")
    if use_bo:
        bo_ext = nc.dram_tensor("boe", [D], F32, kind="ExternalInput")
    out_ext = nc.dram_tensor("out", [T, BC, D], F32, kind="ExternalOutput")

    with tile.TileContext(nc) as tc:
        with (
            tc.tile_pool(name="consts", bufs=1) as consts,
            tc.tile_pool(name="wpool", bufs=2) as wpool,
            tc.tile_pool(name="big", bufs=1) as big,
            tc.tile_pool(name="embp", bufs=1) as embp,
            tc.tile_pool(name="kball", bufs=1) as kballp,
            tc.tile_pool(name="attn", bufs=1) as attnp,
            tc.tile_pool(name="kv", bufs=3) as kvp,
            tc.tile_pool(name="work", bufs=3) as work,
            tc.tile_pool(name="smalls", bufs=3) as smalls,
            tc.tile_pool(name="outp", bufs=2) as outp,
            tc.tile_pool(name="ctxT", bufs=2) as ctxTp,
            tc.tile_pool(name="psum", bufs=8, space="PSUM") as psum,
            tc.tile_pool(name="dram", bufs=1, space="DRAM") as dramp,
        ):
            ctx_dram = dramp.tile([T, BC, D], BF16)
            # ---- constant + weight loads ----
            rw_full = consts.tile([128, T, R * H], F32)
            nc.gpsimd.dma_start(out=rw_full, in_=_bcast_part(rw_ext[:]))
            if use_bq:
                bq_full = consts.tile([128, D], F32)
                nc.gpsimd.dma_start(out=bq_full, in_=_bcast_part(bq_ext[:]))
            if use_bo:
                bo_full = consts.tile([128, D], F32)
                nc.gpsimd.dma_start(out=bo_full, in_=_bcast_part(bo_ext[:]))

            wq_t = wpool.tile([128, KCH, D], BF16, tag="w")
            nc.gpsimd.dma_start(
                out=wq_t, in_=wq_ext.rearrange("(k p) o -> p k o", p=128)
            )
            wk_t = wpool.tile([128, KCH, D], BF16, tag="w")
            nc.gpsimd.dma_start(
                out=wk_t, in_=wk_ext.rearrange("(k p) o -> p k o", p=128)
            )

            # embT[p, k, t*BC+b] = emb[t, b, k*128+p]  (bf16 xbar transposes,
            # split across both HWDGE sequencers to shorten the startup gate)
            embT = embp.tile([128, KCH, T * BC], BF16)
            for k in range(KCH):
                nc.scalar.dma_start_transpose(
                    out=embT[:, k, :],
                    in_=emb_ext[:, :, k * 128 : (k + 1) * 128].rearrange(
                        "t b d -> (t b) d"
                    ),
                )

            qp_all = big.tile([128, T, D], BF16, tag="qpctx")
            kb_all = kballp.tile([128, T, D], BF16)
            scores = attnp.tile([128, T, R * H], F32)
            p2_all = attnp.tile([128, T, R * H], BF16)

            def emit_pair(t, j, r0, extras):
                """scores[t, r0-block] = per-head dot(Qp[t], Kb[j]) via
                bf16 mul + binary-tree halving over dh (dh-major layout:
                folding dh halves == folding contiguous column halves)."""
                prod = work.tile([128, D], BF16, tag="prod")
                nc.vector.tensor_mul(prod, qp_all[:, t, :], kb_all[:, j, :])
                nc.vector.tensor_add(prod[:, 0:512], prod[:, 0:512], prod[:, 512:1024])
                nc.vector.tensor_add(prod[:, 0:256], prod[:, 0:256], prod[:, 256:512])
                sc32 = smalls.tile([128, 128], F32, tag="sc32")
                nc.vector.tensor_add(sc32, prod[:, 0:128], prod[:, 128:256])
                nc.vector.tensor_add(sc32[:, 0:64], sc32[:, 0:64], sc32[:, 64:128])
                nc.vector.tensor_add(sc32[:, 0:32], sc32[:, 0:32], sc32[:, 32:64])
                dst = scores[:, t, r0 * H : (r0 + 1) * H]
                nc.vector.tensor_add(dst, sc32[:, 0:16], sc32[:, 16:32])
                for rx in extras:
                    nc.vector.tensor_copy(scores[:, t, rx * H : (rx + 1) * H], dst)

            def emit_softmax(t):
                s_t = scores[:, t, :]
                nc.vector.tensor_mul(s_t, s_t, rw_full[:, t, :])  # *= rel_w
                m = smalls.tile([128, H], F32, tag="m")
                nc.vector.reduce_max(
                    out=m, in_=s_t.rearrange("p (r h) -> p h r", h=H), axis=AX.X
                )
                e_t = p2_all[:, t, :]
                nc.vector.tensor_sub(
                    e_t.rearrange("p (r h) -> p r h", h=H),
                    s_t.rearrange("p (r h) -> p r h", h=H),
                    _bcast_free(m, R, 0),
                )
                # exp((s - m) / sqrt(DH)); the 1/8 rides ACT's free affine
                nc.scalar.activation(e_t, e_t, AF.Exp, scale=0.125)
                ssum = smalls.tile([128, H], F32, tag="ssum")
                nc.vector.reduce_sum(
                    out=ssum, in_=e_t.rearrange("p (r h) -> p h r", h=H), axis=AX.X
                )
                inv = smalls.tile([128, H], F32, tag="inv")
                nc.vector.reciprocal(inv, ssum)
                nc.vector.tensor_mul(
                    e_t.rearrange("p (r h) -> p r h", h=H),
                    e_t.rearrange("p (r h) -> p r h", h=H),
                    _bcast_free(inv, R, 0),
                )
                nc.vector.tensor_mul(e_t, e_t, rw_full[:, t, :])  # P = attn*rel_w
                for j, r0, extras in groups[t]:
                    for rx in extras:
                        nc.vector.tensor_add(
                            e_t[:, r0 * H : (r0 + 1) * H],
                            e_t[:, r0 * H : (r0 + 1) * H],
                            e_t[:, rx * H : (rx + 1) * H],
                        )

            # ---- phase 1: Q+K projections (dense PE stream) + trailing scores ----
            next_pair = 0
            for t in range(T):
                psq0 = psum.tile([128, 512], F32, tag="ps", name="ps")
                psq1 = psum.tile([128, 512], F32, tag="ps", name="ps")
                psk0 = psum.tile([128, 512], F32, tag="ps", name="ps")
                psk1 = psum.tile([128, 512], F32, tag="ps", name="ps")
                for k in range(KCH):
                    lhs = embT[:, k, t * BC : (t + 1) * BC]
                    fl = dict(start=(k == 0), stop=(k == KCH - 1))
                    nc.tensor.matmul(psq0, lhs, wq_t[:, k, 0:512], **fl)
                    nc.tensor.matmul(psq1, lhs, wq_t[:, k, 512:1024], **fl)
                    nc.tensor.matmul(psk0, lhs, wk_t[:, k, 0:512], **fl)
                    nc.tensor.matmul(psk1, lhs, wk_t[:, k, 512:1024], **fl)
                nc.scalar.copy(out=qp_all[:, t, 0:512], in_=psq0)
                nc.scalar.copy(out=qp_all[:, t, 512:1024], in_=psq1)
                nc.scalar.copy(out=kb_all[:, t, 0:512], in_=psk0)
                nc.scalar.copy(out=kb_all[:, t, 512:1024], in_=psk1)
                if use_bq:
                    nc.vector.tensor_add(qp_all[:, t, :], qp_all[:, t, :], bq_full)
                # emit score pairs that just became available
                while next_pair < len(pair_sched) and pair_sched[next_pair][0] <= t:
                    _, tp, jp, r0p, exp_ = pair_sched[next_pair]
                    emit_pair(tp, jp, r0p, exp_)
                    if last_pair_of_t[tp] == next_pair:
                        emit_softmax(tp)
                    next_pair += 1

            wv_t = wpool.tile([128, KCH, D], BF16, tag="w")  # reuses wq slot
            nc.gpsimd.dma_start(
                out=wv_t, in_=wv_ext.rearrange("(k p) o -> p k o", p=128)
            )
            wo_t = wpool.tile([128, KCH, D], BF16, tag="w")  # reuses wk slot
            nc.gpsimd.dma_start(
                out=wo_t, in_=wo_ext.rearrange("(k p) o -> p k o", p=128)
            )

            # ---- phase 2: V projections (recomputed) + ctx accumulation ----
            ctx_all = big.tile([128, T, D], BF16, tag="qpctx")  # reuses qp slot
            done_t = set()
            alt = 0
            for j in range(T):
                psv0 = psum.tile([128, 512], F32, tag="ps", name="ps")
                psv1 = psum.tile([128, 512], F32, tag="ps", name="ps")
                for k in range(KCH):
                    lhs = embT[:, k, j * BC : (j + 1) * BC]
                    fl = dict(start=(k == 0), stop=(k == KCH - 1))
                    nc.tensor.matmul(psv0, lhs, wv_t[:, k, 0:512], **fl)
                    nc.tensor.matmul(psv1, lhs, wv_t[:, k, 512:1024], **fl)
                vb = kvp.tile([128, D], BF16, tag="vb")
                nc.scalar.copy(out=vb[:, 0:512], in_=psv0)
                nc.scalar.copy(out=vb[:, 512:1024], in_=psv1)
                for t, r0 in pairs_by_j[j]:
                    pb = _bcast_free(p2_all[:, t, r0 * H : (r0 + 1) * H], DH, 0)
                    vb3 = vb.rearrange("p (dh h) -> p dh h", h=H)
                    c3 = ctx_all[:, t, :].rearrange("p (dh h) -> p dh h", h=H)
                    if t not in done_t:
                        done_t.add(t)
                        nc.vector.tensor_mul(c3, vb3, pb)
                    else:
                        tmp = work.tile([128, D], BF16, tag="ctmp")
                        nc.vector.tensor_mul(
                            tmp.rearrange("p (dh h) -> p dh h", h=H), vb3, pb
                        )
                        eng = nc.vector if alt % 2 == 0 else nc.gpsimd
                        alt += 1
                        eng.tensor_add(ctx_all[:, t, :], ctx_all[:, t, :], tmp)
                # ship finished ctx rows to DRAM scratch (for xbar re-transpose)
                for t in range(T):
                    if last_j[t] == j:
                        nc.sync.dma_start(out=ctx_dram[t], in_=ctx_all[:, t, :])

            # ---- phase 3: output projection ----
            TG = 2  # t-group size for transposing loads
            for tg in range(T // TG):
                ctxT = ctxTp.tile([128, KCH, TG * BC], BF16)
                for k in range(KCH):
                    nc.scalar.dma_start_transpose(
                        out=ctxT[:, k, :],
                        in_=ctx_dram[
                            tg * TG : (tg + 1) * TG, :, k * 128 : (k + 1) * 128
                        ].rearrange("t b d -> (t b) d"),
                    )
                for ti in range(TG):
                    t = tg * TG + ti
                    o_t = outp.tile([128, D], F32)
                    pso0 = psum.tile([128, 512], F32, tag="ps", name="ps")
                    pso1 = psum.tile([128, 512], F32, tag="ps", name="ps")
                    for k in range(KCH):
                        lhs = ctxT[:, k, ti * BC : (ti + 1) * BC]
                        fl = dict(start=(k == 0), stop=(k == KCH - 1))
                        nc.tensor.matmul(pso0, lhs, wo_t[:, k, 0:512], **fl)
                        nc.tensor.matmul(pso1, lhs, wo_t[:, k, 512:1024], **fl)
                    nc.scalar.copy(out=o_t[:, 0:512], in_=pso0)
                    nc.scalar.copy(out=o_t[:, 512:1024], in_=pso1)
                    if use_bo:
                        nc.vector.tensor_add(o_t, o_t, bo_full)
                    nc.sync.dma_start(out=out_ext[t], in_=o_t)

    return nc


_CACHE = {}


def _get_program(rel_idx, use_bq, use_bo):
    key = (rel_idx.tobytes(), use_bq, use_bo)
    if key not in _CACHE:
        nc = _build(rel_idx, use_bq, use_bo)
        nc.finalize()  # runs the bacc passes (reg alloc, wait lowering, ...)
        _CACHE[key] = nc
    return _CACHE[key]


def kernel(
    table_embs,
    rel_embs,
    rel_idx,
    Wq,
    bq,
    Wk,
    bk,
    Wv,
    bv,
    Wo,
    bo,
    w_rel,
    b_rel,
    _trace=False,
):
    table_embs = np.asarray(table_embs, dtype=np.float32)
    rel_embs = np.asarray(rel_embs, dtype=np.float32)
    rel_idx = np.asarray(rel_idx).astype(np.int64)
    Wq, Wk, Wv, Wo = (np.asarray(w, dtype=np.float32) for w in (Wq, Wk, Wv, Wo))
    bq, bk, bv, bo = (np.asarray(b, dtype=np.float32) for b in (bq, bk, bv, bo))
    w_rel = np.asarray(w_rel, dtype=np.float32)
    b_rel = np.asarray(b_rel, dtype=np.float32)

    # ---- host-side tiny prep ----
    rw = 1.0 / (1.0 + np.exp(-(rel_embs @ w_rel + b_rel[0])))  # [T, R] fp32
    rw_full = np.repeat(rw.astype(np.float32), H, axis=1)  # [T, R*H], col=r*16+h
    bf = ml_dtypes.bfloat16
    wq_p = np.ascontiguousarray(Wq.T[:, _PERM], dtype=bf)
    wk_p = np.ascontiguousarray(Wk.T[:, _PERM], dtype=bf)
    wv_p = np.ascontiguousarray(Wv.T[:, _PERM], dtype=bf)
    wo_p = np.ascontiguousarray(Wo.T[_PERM, :], dtype=bf)
    use_bq = bool(np.any(bq))
    bo_eff = Wo @ bv + bo
    use_bo = bool(np.any(bo_eff))
    bq_p = np.ascontiguousarray(bq[_PERM], dtype=np.float32)

    nc = _get_program(rel_idx, use_bq, use_bo)

    in_maps = []
    for c in range(NCORES):
        m = {
            "emb": np.ascontiguousarray(
                table_embs[:, c * BC : (c + 1) * BC, :], dtype=bf
            ),
            "wq": wq_p,
            "wk": wk_p,
            "wv": wv_p,
            "wo": wo_p,
            "rw": rw_full,
        }
        if use_bq:
            m["bqp"] = bq_p
        if use_bo:
            m["boe"] = bo_eff.astype(np.float32)
        in_maps.append(m)

    res = run_bass_kernel_spmd(nc, in_maps, list(range(NCORES)), trace=_trace)
    out = np.empty((T, B, D), dtype=np.float32)
    for c in range(NCORES):
        out[:, c * BC : (c + 1) * BC, :] = res.results[c]["out"]
    if _trace:
        kernel._last_results = res
    return out



# revision 6
# speedup vs baseline: 1.0859x; 1.0859x over previous
"""CrossTableAttention Trainium2 kernel (8-core SPMD, batch-sharded).

Math (per table t, row b):
  rel_w[t,r]   = sigmoid(rel_embs[t,r] . w_rel + b_rel)          (host, tiny)
  Qp[t]        = emb[t] @ Wq.T (+bq)                              [B, D]
  Kb[j]        = emb[j] @ Wk.T        (bias bk is softmax-invariant -> dropped)
  Vb[j]        = emb[j] @ Wv.T        (bias bv folded into output bias)
  score[t,b,h,r] = rel_w[t,r] * (Qp[t,b,h,:] . Kb[j_r,b,h,:]) / sqrt(DH)
  attn         = softmax_r(score);  P = attn * rel_w
  ctx[t,b]     = sum_r P[t,b,h,r] * Vb[j_r,b,h,:]
  out[t]       = ctx[t] @ Wo.T + (Wo @ bv + bo)

Key algebraic optimization: K/V projections are computed per *table* (16) instead
of per (table, relation) gather (128) - the relation weight is a scalar that
commutes with the linear projection.  5x fewer matmul FLOPs than the reference.

Device layout notes:
 - Activations live as [row(b) on partitions, feature on free] so the attention
   inner products reduce along the free axis (DVE).
 - q/k/v/ctx feature axis is permuted to dh-major (f = dh*16 + h) so the
   per-(b,h) attention-weight broadcast has a step-1 innermost AP dim
   (keeps DVE tensor_tensor in 2x bf16 mode).  Weight matrices are permuted
   host-side to produce/consume this layout directly.
 - scores/P layout per t: [128 b, 128 = r*16 + h] (h innermost).
 - Matmul operands need the contraction dim (d) on partitions; embT is produced
   by bf16 DMA-xbar transposing loads straight from the (host pre-cast) input.
"""

import sys

sys.path.insert(0, "/opt/trn_rl_repo")

import numpy as np
import ml_dtypes

import concourse.bass as bass
import concourse.bacc as bacc_mod
import concourse.mybir as mybir
import concourse.tile as tile
from concourse.bass_utils import run_bass_kernel_spmd

T, B, D, R, H = 16, 1024, 1024, 8, 16
DH = D // H  # 64
NCORES = 8
BC = B // NCORES  # 128 rows per core
KCH = D // 128  # 8 contraction chunks

F32 = mybir.dt.float32
BF16 = mybir.dt.bfloat16
AX = mybir.AxisListType
AF = mybir.ActivationFunctionType

# feature permutation: new f = dh*16 + h  <->  old o = h*64 + dh
_PERM = np.array([(f % H) * DH + f // H for f in range(D)], dtype=np.int64)


def _bcast_free(ap, n, pos):
    """Insert a [step=0, n] broadcast dim into an AP's free dims at `pos`
    (pos counts free dims, 0 = outermost free dim)."""
    new = list(ap.ap)
    new.insert(1 + pos, [0, n])
    return bass.AP(tensor=ap.tensor, offset=ap.offset, ap=new)


def _bcast_part(ap, n=128):
    """Broadcast a (DRAM) AP across n partitions by prepending a [0, n] dim."""
    return bass.AP(tensor=ap.tensor, offset=ap.offset, ap=[[0, n]] + list(ap.ap))


def _structure(rel_idx):
    """Host-side dedup of the (t, j) gather structure."""
    groups = []  # per t: list of (j, r0, [extra r's])
    for t in range(T):
        by_j = {}
        for r in range(R):
            by_j.setdefault(int(rel_idx[t, r]), []).append(r)
        groups.append([(j, rs[0], rs[1:]) for j, rs in sorted(by_j.items())])
    pairs_by_j = {j: [] for j in range(T)}
    for t in range(T):
        for j, r0, extras in groups[t]:
            pairs_by_j[j].append((t, r0))
    # first j (in ascending j processing order) touching each t
    first_j = {}
    for j in range(T):
        for t, _ in pairs_by_j[j]:
            if t not in first_j:
                first_j[t] = j
    # last j touching each t (to schedule ctx output DMA)
    last_j = {}
    for j in range(T):
        for t, _ in pairs_by_j[j]:
            last_j[t] = j
    return groups, pairs_by_j, first_j, last_j


def _build(rel_idx, use_bq, use_bo):
    """Build the SPMD bass program (identical on all cores).

    v2 structure (from v1 trace analysis):
     - Q+K projections interleaved per t (shared stationary embT chunk ->
       LDWEIGHTS amortized over 4 matmuls), Kb fully SBUF-resident so the
       scores phase never back-pressures the PE.
     - score pairs emitted in availability order (sorted by max(t, j)) so the
       DVE trails the projection stream without stalls.
     - per-head dot products via a binary tree of tensor_tensor adds (bf16 2x
       mode for the large levels, fp32 tail) instead of 1x tensor_reduce.
     - V projections recomputed in the ctx phase (keeps PE busy there);
       ctx accumulation adds alternate DVE / GPSIMD.
    """
    groups, pairs_by_j, first_j, last_j = _structure(rel_idx)

    # (t, j, r0, extras) sorted by when both Qp[t] and Kb[j] become available
    pair_sched = []
    for t in range(T):
        for j, r0, extras in groups[t]:
            pair_sched.append((max(t, j), t, j, r0, extras))
    pair_sched.sort(key=lambda x: (x[0], x[1]))
    last_pair_of_t = {}
    for i, (_, t, j, r0, ex) in enumerate(pair_sched):
        last_pair_of_t[t] = i

    nc = bacc_mod.Bacc(None, target_bir_lowering=False, debug=False)
    emb_ext = nc.dram_tensor("emb", [T, BC, D], BF16, kind="ExternalInput")
    wq_ext = nc.dram_tensor("wq", [D, D], BF16, kind="ExternalInput")
    wk_ext = nc.dram_tensor("wk", [D, D], BF16, kind="ExternalInput")
    wv_ext = nc.dram_tensor("wv", [D, D], BF16, kind="ExternalInput")
    wo_ext = nc.dram_tensor("wo", [D, D], BF16, kind="ExternalInput")
    rw_ext = nc.dram_tensor("rw", [T, R * H], F32, kind="ExternalInput")
    id_ext = nc.dram_tensor("ident", [128, 128], BF16, kind="ExternalInput")
    if use_bq:
        bq_ext = nc.dram_tensor("bqp", [D], F32, kind="ExternalInput")
    if use_bo:
        bo_ext = nc.dram_tensor("boe", [D], F32, kind="ExternalInput")
    out_ext = nc.dram_tensor("out", [T, BC, D], F32, kind="ExternalOutput")

    with tile.TileContext(nc) as tc:
        with (
            tc.tile_pool(name="consts", bufs=1) as consts,
            tc.tile_pool(name="wpool", bufs=2) as wpool,
            tc.tile_pool(name="big", bufs=1) as big,
            tc.tile_pool(name="embp", bufs=1) as embp,
            tc.tile_pool(name="kball", bufs=1) as kballp,
            tc.tile_pool(name="attn", bufs=1) as attnp,
            tc.tile_pool(name="kv", bufs=3) as kvp,
            tc.tile_pool(name="work", bufs=3) as work,
            tc.tile_pool(name="smalls", bufs=3) as smalls,
            tc.tile_pool(name="outp", bufs=2) as outp,
            tc.tile_pool(name="ctxT", bufs=2) as ctxTp,
            tc.tile_pool(name="psum", bufs=6, space="PSUM") as psum,
            tc.tile_pool(name="psumt", bufs=2, space="PSUM") as psumt,
        ):
            # ---- constant + weight loads ----
            ident = consts.tile([128, 128], BF16)
            nc.gpsimd.dma_start(out=ident, in_=id_ext[:])
            rw_full = consts.tile([128, T, R * H], F32)
            nc.gpsimd.dma_start(out=rw_full, in_=_bcast_part(rw_ext[:]))
            if use_bq:
                bq_full = consts.tile([128, D], F32)
                nc.gpsimd.dma_start(out=bq_full, in_=_bcast_part(bq_ext[:]))
            if use_bo:
                bo_full = consts.tile([128, D], F32)
                nc.gpsimd.dma_start(out=bo_full, in_=_bcast_part(bo_ext[:]))

            wq_t = wpool.tile([128, KCH, D], BF16, tag="w")
            nc.gpsimd.dma_start(
                out=wq_t, in_=wq_ext.rearrange("(k p) o -> p k o", p=128)
            )
            wk_t = wpool.tile([128, KCH, D], BF16, tag="w")
            nc.gpsimd.dma_start(
                out=wk_t, in_=wk_ext.rearrange("(k p) o -> p k o", p=128)
            )

            # embT[p, k, t*BC+b] = emb[t, b, k*128+p]  (bf16 xbar transposes,
            # split across both HWDGE sequencers to shorten the startup gate)
            embT = embp.tile([128, KCH, T * BC], BF16)
            for k in range(KCH):
                nc.scalar.dma_start_transpose(
                    out=embT[:, k, :],
                    in_=emb_ext[:, :, k * 128 : (k + 1) * 128].rearrange(
                        "t b d -> (t b) d"
                    ),
                )

            qp_all = big.tile([128, T, D], BF16, tag="qpctx")
            kb_all = kballp.tile([128, T, D], BF16)
            scores = attnp.tile([128, T, R * H], F32)
            p2_all = attnp.tile([128, T, R * H], BF16)

            def emit_pair(t, j, r0, extras):
                """scores[t, r0-block] = per-head dot(Qp[t], Kb[j]) via
                bf16 mul + binary-tree halving over dh (dh-major layout:
                folding dh halves == folding contiguous column halves)."""
                prod = work.tile([128, D], BF16, tag="prod")
                nc.vector.tensor_mul(prod, qp_all[:, t, :], kb_all[:, j, :])
                nc.vector.tensor_add(prod[:, 0:512], prod[:, 0:512], prod[:, 512:1024])
                nc.vector.tensor_add(prod[:, 0:256], prod[:, 0:256], prod[:, 256:512])
                sc32 = smalls.tile([128, 128], F32, tag="sc32")
                nc.vector.tensor_add(sc32, prod[:, 0:128], prod[:, 128:256])
                nc.vector.tensor_add(sc32[:, 0:64], sc32[:, 0:64], sc32[:, 64:128])
                nc.vector.tensor_add(sc32[:, 0:32], sc32[:, 0:32], sc32[:, 32:64])
                dst = scores[:, t, r0 * H : (r0 + 1) * H]
                nc.vector.tensor_add(dst, sc32[:, 0:16], sc32[:, 16:32])
                for rx in extras:
                    nc.vector.tensor_copy(scores[:, t, rx * H : (rx + 1) * H], dst)

            def emit_softmax(t):
                s_t = scores[:, t, :]
                nc.vector.tensor_mul(s_t, s_t, rw_full[:, t, :])  # *= rel_w
                m = smalls.tile([128, H], F32, tag="m")
                nc.vector.reduce_max(
                    out=m, in_=s_t.rearrange("p (r h) -> p h r", h=H), axis=AX.X
                )
                e_t = p2_all[:, t, :]
                nc.vector.tensor_sub(
                    e_t.rearrange("p (r h) -> p r h", h=H),
                    s_t.rearrange("p (r h) -> p r h", h=H),
                    _bcast_free(m, R, 0),
                )
                # exp((s - m) / sqrt(DH)); the 1/8 rides ACT's free affine
                nc.scalar.activation(e_t, e_t, AF.Exp, scale=0.125)
                ssum = smalls.tile([128, H], F32, tag="ssum")
                nc.vector.reduce_sum(
                    out=ssum, in_=e_t.rearrange("p (r h) -> p h r", h=H), axis=AX.X
                )
                inv = smalls.tile([128, H], F32, tag="inv")
                nc.vector.reciprocal(inv, ssum)
                nc.vector.tensor_mul(
                    e_t.rearrange("p (r h) -> p r h", h=H),
                    e_t.rearrange("p (r h) -> p r h", h=H),
                    _bcast_free(inv, R, 0),
                )
                nc.vector.tensor_mul(e_t, e_t, rw_full[:, t, :])  # P = attn*rel_w
                for j, r0, extras in groups[t]:
                    for rx in extras:
                        nc.vector.tensor_add(
                            e_t[:, r0 * H : (r0 + 1) * H],
                            e_t[:, r0 * H : (r0 + 1) * H],
                            e_t[:, rx * H : (rx + 1) * H],
                        )

            # ---- phase 1: Q+K projections (dense PE stream) + trailing scores ----
            next_pair = 0
            for t in range(T):
                psq0 = psum.tile([128, 512], F32, tag="ps", name="ps")
                psq1 = psum.tile([128, 512], F32, tag="ps", name="ps")
                psk0 = psum.tile([128, 512], F32, tag="ps", name="ps")
                psk1 = psum.tile([128, 512], F32, tag="ps", name="ps")
                for k in range(KCH):
                    lhs = embT[:, k, t * BC : (t + 1) * BC]
                    fl = dict(start=(k == 0), stop=(k == KCH - 1))
                    nc.tensor.matmul(psq0, lhs, wq_t[:, k, 0:512], **fl)
                    nc.tensor.matmul(psq1, lhs, wq_t[:, k, 512:1024], **fl)
                    nc.tensor.matmul(psk0, lhs, wk_t[:, k, 0:512], **fl)
                    nc.tensor.matmul(psk1, lhs, wk_t[:, k, 512:1024], **fl)
                nc.scalar.copy(out=qp_all[:, t, 0:512], in_=psq0)
                nc.scalar.copy(out=qp_all[:, t, 512:1024], in_=psq1)
                nc.scalar.copy(out=kb_all[:, t, 0:512], in_=psk0)
                nc.scalar.copy(out=kb_all[:, t, 512:1024], in_=psk1)
                if use_bq:
                    nc.vector.tensor_add(qp_all[:, t, :], qp_all[:, t, :], bq_full)
                # emit score pairs that just became available
                while next_pair < len(pair_sched) and pair_sched[next_pair][0] <= t:
                    _, tp, jp, r0p, exp_ = pair_sched[next_pair]
                    emit_pair(tp, jp, r0p, exp_)
                    if last_pair_of_t[tp] == next_pair:
                        emit_softmax(tp)
                    next_pair += 1

            wv_t = wpool.tile([128, KCH, D], BF16, tag="w")  # reuses wq slot
            nc.gpsimd.dma_start(
                out=wv_t, in_=wv_ext.rearrange("(k p) o -> p k o", p=128)
            )
            wo_t = wpool.tile([128, KCH, D], BF16, tag="w")  # reuses wk slot
            nc.gpsimd.dma_start(
                out=wo_t, in_=wo_ext.rearrange("(k p) o -> p k o", p=128)
            )

            # ---- phase 2: V projections (recomputed) + ctx accumulation ----
            ctx_all = big.tile([128, T, D], BF16, tag="qpctx")  # reuses qp slot

            def emit_out_proj(t):
                """ctx[t] (SBUF, b-part) -> PE transpose -> Wo matmuls -> out DMA.
                Runs inline as soon as ctx[t] is complete, overlapping the
                DVE-bound ctx phase instead of a serial phase 3."""
                ctxT_ps = psumt.tile([128, KCH, 128], BF16, tag="tr", name="tr")
                for k in range(KCH):
                    nc.tensor.transpose(
                        out=ctxT_ps[:, k, :],
                        in_=ctx_all[:, t, k * 128 : (k + 1) * 128],
                        identity=ident,
                    )
                ctxT_sb = ctxTp.tile([128, KCH, 128], BF16)
                nc.scalar.copy(out=ctxT_sb, in_=ctxT_ps)
                o_t = outp.tile([128, D], F32)
                pso0 = psum.tile([128, 512], F32, tag="ps", name="ps")
                pso1 = psum.tile([128, 512], F32, tag="ps", name="ps")
                for k in range(KCH):
                    lhs = ctxT_sb[:, k, :]
                    fl = dict(start=(k == 0), stop=(k == KCH - 1))
                    nc.tensor.matmul(pso0, lhs, wo_t[:, k, 0:512], **fl)
                    nc.tensor.matmul(pso1, lhs, wo_t[:, k, 512:1024], **fl)
                nc.scalar.copy(out=o_t[:, 0:512], in_=pso0)
                nc.scalar.copy(out=o_t[:, 512:1024], in_=pso1)
                if use_bo:
                    nc.vector.tensor_add(o_t, o_t, bo_full)
                nc.sync.dma_start(out=out_ext[t], in_=o_t)

            done_t = set()
            alt = 0
            for j in range(T):
                psv0 = psum.tile([128, 512], F32, tag="ps", name="ps")
                psv1 = psum.tile([128, 512], F32, tag="ps", name="ps")
                for k in range(KCH):
                    lhs = embT[:, k, j * BC : (j + 1) * BC]
                    fl = dict(start=(k == 0), stop=(k == KCH - 1))
                    nc.tensor.matmul(psv0, lhs, wv_t[:, k, 0:512], **fl)
                    nc.tensor.matmul(psv1, lhs, wv_t[:, k, 512:1024], **fl)
                vb = kvp.tile([128, D], BF16, tag="vb")
                nc.scalar.copy(out=vb[:, 0:512], in_=psv0)
                nc.scalar.copy(out=vb[:, 512:1024], in_=psv1)
                for t, r0 in pairs_by_j[j]:
                    pb = _bcast_free(p2_all[:, t, r0 * H : (r0 + 1) * H], DH, 0)
                    vb3 = vb.rearrange("p (dh h) -> p dh h", h=H)
                    c3 = ctx_all[:, t, :].rearrange("p (dh h) -> p dh h", h=H)
                    if t not in done_t:
                        done_t.add(t)
                        nc.vector.tensor_mul(c3, vb3, pb)
                    else:
                        tmp = work.tile([128, D], BF16, tag="ctmp")
                        nc.vector.tensor_mul(
                            tmp.rearrange("p (dh h) -> p dh h", h=H), vb3, pb
                        )
                        eng = nc.vector if alt % 2 == 0 else nc.gpsimd
                        alt += 1
                        eng.tensor_add(ctx_all[:, t, :], ctx_all[:, t, :], tmp)
                # project + ship finished ctx rows inline
                for t in range(T):
                    if last_j[t] == j:
                        emit_out_proj(t)

    return nc


_CACHE = {}


def _get_program(rel_idx, use_bq, use_bo):
    key = (rel_idx.tobytes(), use_bq, use_bo)
    if key not in _CACHE:
        nc = _build(rel_idx, use_bq, use_bo)
        nc.finalize()  # runs the bacc passes (reg alloc, wait lowering, ...)
        _CACHE[key] = nc
    return _CACHE[key]


def kernel(
    table_embs,
    rel_embs,
    rel_idx,
    Wq,
    bq,
    Wk,
    bk,
    Wv,
    bv,
    Wo,
    bo,
    w_rel,
    b_rel,
    _trace=False,
):
    table_embs = np.asarray(table_embs, dtype=np.float32)
    rel_embs = np.asarray(rel_embs, dtype=np.float32)
    rel_idx = np.asarray(rel_idx).astype(np.int64)
    Wq, Wk, Wv, Wo = (np.asarray(w, dtype=np.float32) for w in (Wq, Wk, Wv, Wo))
    bq, bk, bv, bo = (np.asarray(b, dtype=np.float32) for b in (bq, bk, bv, bo))
    w_rel = np.asarray(w_rel, dtype=np.float32)
    b_rel = np.asarray(b_rel, dtype=np.float32)

    # ---- host-side tiny prep ----
    rw = 1.0 / (1.0 + np.exp(-(rel_embs @ w_rel + b_rel[0])))  # [T, R] fp32
    rw_full = np.repeat(rw.astype(np.float32), H, axis=1)  # [T, R*H], col=r*16+h
    bf = ml_dtypes.bfloat16
    wq_p = np.ascontiguousarray(Wq.T[:, _PERM], dtype=bf)
    wk_p = np.ascontiguousarray(Wk.T[:, _PERM], dtype=bf)
    wv_p = np.ascontiguousarray(Wv.T[:, _PERM], dtype=bf)
    wo_p = np.ascontiguousarray(Wo.T[_PERM, :], dtype=bf)
    use_bq = bool(np.any(bq))
    bo_eff = Wo @ bv + bo
    use_bo = bool(np.any(bo_eff))
    bq_p = np.ascontiguousarray(bq[_PERM], dtype=np.float32)

    nc = _get_program(rel_idx, use_bq, use_bo)

    in_maps = []
    for c in range(NCORES):
        m = {
            "emb": np.ascontiguousarray(
                table_embs[:, c * BC : (c + 1) * BC, :], dtype=bf
            ),
            "wq": wq_p,
            "wk": wk_p,
            "wv": wv_p,
            "wo": wo_p,
            "rw": rw_full,
            "ident": np.eye(128, dtype=bf),
        }
        if use_bq:
            m["bqp"] = bq_p
        if use_bo:
            m["boe"] = bo_eff.astype(np.float32)
        in_maps.append(m)

    res = run_bass_kernel_spmd(nc, in_maps, list(range(NCORES)), trace=_trace)
    out = np.empty((T, B, D), dtype=np.float32)
    for c in range(NCORES):
        out[:, c * BC : (c + 1) * BC, :] = res.results[c]["out"]
    if _trace:
        kernel._last_results = res
    return out



# revision 8
# speedup vs baseline: 1.2669x; 1.1667x over previous
"""CrossTableAttention Trainium2 kernel (8-core SPMD, batch-sharded).

Math (per table t, row b):
  rel_w[t,r]   = sigmoid(rel_embs[t,r] . w_rel + b_rel)          (host, tiny)
  Qp[t]        = emb[t] @ Wq.T (+bq)                              [B, D]
  Kb[j]        = emb[j] @ Wk.T        (bias bk is softmax-invariant -> dropped)
  Vb[j]        = emb[j] @ Wv.T        (bias bv folded into output bias)
  score[t,b,h,r] = rel_w[t,r] * (Qp[t,b,h,:] . Kb[j_r,b,h,:]) / sqrt(DH)
  attn         = softmax_r(score);  P = attn * rel_w
  ctx[t,b]     = sum_r P[t,b,h,r] * Vb[j_r,b,h,:]
  out[t]       = ctx[t] @ Wo.T + (Wo @ bv + bo)

Key algebraic optimization: K/V projections are computed per *table* (16) instead
of per (table, relation) gather (128) - the relation weight is a scalar that
commutes with the linear projection.  5x fewer matmul FLOPs than the reference.

Device layout notes:
 - Activations live as [row(b) on partitions, feature on free] so the attention
   inner products reduce along the free axis (DVE).
 - q/k/v/ctx feature axis is permuted to dh-major (f = dh*16 + h) so the
   per-(b,h) attention-weight broadcast has a step-1 innermost AP dim
   (keeps DVE tensor_tensor in 2x bf16 mode).  Weight matrices are permuted
   host-side to produce/consume this layout directly.
 - scores/P layout per t: [128 b, 128 = r*16 + h] (h innermost).
 - Matmul operands need the contraction dim (d) on partitions; embT is produced
   by bf16 DMA-xbar transposing loads straight from the (host pre-cast) input.
"""

import sys

sys.path.insert(0, "/opt/trn_rl_repo")

import numpy as np
import ml_dtypes

import concourse.bass as bass
import concourse.bacc as bacc_mod
import concourse.mybir as mybir
import concourse.tile as tile
from concourse.bass_utils import run_bass_kernel_spmd

T, B, D, R, H = 16, 1024, 1024, 8, 16
DH = D // H  # 64
NCORES = 8
BC = B // NCORES  # 128 rows per core
KCH = D // 128  # 8 contraction chunks

F32 = mybir.dt.float32
BF16 = mybir.dt.bfloat16
AX = mybir.AxisListType
AF = mybir.ActivationFunctionType

# feature permutation: new f = dh*16 + h  <->  old o = h*64 + dh
_PERM = np.array([(f % H) * DH + f // H for f in range(D)], dtype=np.int64)


def _bcast_free(ap, n, pos):
    """Insert a [step=0, n] broadcast dim into an AP's free dims at `pos`
    (pos counts free dims, 0 = outermost free dim)."""
    new = list(ap.ap)
    new.insert(1 + pos, [0, n])
    return bass.AP(tensor=ap.tensor, offset=ap.offset, ap=new)


def _bcast_part(ap, n=128):
    """Broadcast a (DRAM) AP across n partitions by prepending a [0, n] dim."""
    return bass.AP(tensor=ap.tensor, offset=ap.offset, ap=[[0, n]] + list(ap.ap))


def _structure(rel_idx):
    """Host-side dedup of the (t, j) gather structure."""
    groups = []  # per t: list of (j, r0, [extra r's])
    for t in range(T):
        by_j = {}
        for r in range(R):
            by_j.setdefault(int(rel_idx[t, r]), []).append(r)
        groups.append([(j, rs[0], rs[1:]) for j, rs in sorted(by_j.items())])
    pairs_by_j = {j: [] for j in range(T)}
    for t in range(T):
        for j, r0, extras in groups[t]:
            pairs_by_j[j].append((t, r0))
    # first j (in ascending j processing order) touching each t
    first_j = {}
    for j in range(T):
        for t, _ in pairs_by_j[j]:
            if t not in first_j:
                first_j[t] = j
    # last j touching each t (to schedule ctx output DMA)
    last_j = {}
    for j in range(T):
        for t, _ in pairs_by_j[j]:
            last_j[t] = j
    return groups, pairs_by_j, first_j, last_j


def _build(rel_idx, use_bq, use_bo):
    """Build the SPMD bass program (identical on all cores).

    v2 structure (from v1 trace analysis):
     - Q+K projections interleaved per t (shared stationary embT chunk ->
       LDWEIGHTS amortized over 4 matmuls), Kb fully SBUF-resident so the
       scores phase never back-pressures the PE.
     - score pairs emitted in availability order (sorted by max(t, j)) so the
       DVE trails the projection stream without stalls.
     - per-head dot products via a binary tree of tensor_tensor adds (bf16 2x
       mode for the large levels, fp32 tail) instead of 1x tensor_reduce.
     - V projections recomputed in the ctx phase (keeps PE busy there);
       ctx accumulation adds alternate DVE / GPSIMD.
    """
    groups, pairs_by_j, first_j, last_j = _structure(rel_idx)

    # (t, j, r0, extras) sorted by when both Qp[t] and Kb[j] become available
    pair_sched = []
    for t in range(T):
        for j, r0, extras in groups[t]:
            pair_sched.append((max(t, j), t, j, r0, extras))
    pair_sched.sort(key=lambda x: (x[0], x[1]))
    last_pair_of_t = {}
    for i, (_, t, j, r0, ex) in enumerate(pair_sched):
        last_pair_of_t[t] = i

    nc = bacc_mod.Bacc(None, target_bir_lowering=False, debug=False)
    emb_ext = nc.dram_tensor("emb", [T, BC, D], BF16, kind="ExternalInput")
    wq_ext = nc.dram_tensor("wq", [D, D], BF16, kind="ExternalInput")
    wk_ext = nc.dram_tensor("wk", [D, D], BF16, kind="ExternalInput")
    wv_ext = nc.dram_tensor("wv", [D, D], BF16, kind="ExternalInput")
    wo_ext = nc.dram_tensor("wo", [D, D], BF16, kind="ExternalInput")
    rw_ext = nc.dram_tensor("rw", [T, R * H], F32, kind="ExternalInput")
    id_ext = nc.dram_tensor("ident", [128, 128], BF16, kind="ExternalInput")
    if use_bq:
        bq_ext = nc.dram_tensor("bqp", [D], F32, kind="ExternalInput")
    if use_bo:
        bo_ext = nc.dram_tensor("boe", [D], F32, kind="ExternalInput")
    out_ext = nc.dram_tensor("out", [T, BC, D], F32, kind="ExternalOutput")

    with tile.TileContext(nc) as tc:
        with (
            tc.tile_pool(name="consts", bufs=1) as consts,
            tc.tile_pool(name="wpool", bufs=2) as wpool,
            tc.tile_pool(name="big", bufs=1) as big,
            tc.tile_pool(name="embp", bufs=1) as embp,
            tc.tile_pool(name="kball", bufs=1) as kballp,
            tc.tile_pool(name="vball", bufs=1) as vballp,
            tc.tile_pool(name="attn", bufs=1) as attnp,
            tc.tile_pool(name="work", bufs=3) as work,
            tc.tile_pool(name="smalls", bufs=3) as smalls,
            tc.tile_pool(name="outp", bufs=2) as outp,
            tc.tile_pool(name="ctxT", bufs=2) as ctxTp,
            tc.tile_pool(name="psum", bufs=6, space="PSUM") as psum,
            tc.tile_pool(name="psumt", bufs=2, space="PSUM") as psumt,
        ):
            # ---- constant + weight loads ----
            ident = consts.tile([128, 128], BF16)
            nc.gpsimd.dma_start(out=ident, in_=id_ext[:])
            rw_full = consts.tile([128, T, R * H], F32)
            nc.gpsimd.dma_start(out=rw_full, in_=_bcast_part(rw_ext[:]))
            if use_bq:
                bq_full = consts.tile([128, D], F32)
                nc.gpsimd.dma_start(out=bq_full, in_=_bcast_part(bq_ext[:]))
            if use_bo:
                bo_full = consts.tile([128, D], F32)
                nc.gpsimd.dma_start(out=bo_full, in_=_bcast_part(bo_ext[:]))

            wq_t = wpool.tile([128, KCH, D], BF16, tag="w")
            nc.gpsimd.dma_start(
                out=wq_t, in_=wq_ext.rearrange("(k p) o -> p k o", p=128)
            )
            wk_t = wpool.tile([128, KCH, D], BF16, tag="w")
            nc.gpsimd.dma_start(
                out=wk_t, in_=wk_ext.rearrange("(k p) o -> p k o", p=128)
            )

            # embT[p, k, t*BC+b] = emb[t, b, k*128+p]  (bf16 xbar transposes,
            # split across both HWDGE sequencers to shorten the startup gate)
            embT = embp.tile([128, KCH, T * BC], BF16)
            for k in range(KCH):
                nc.scalar.dma_start_transpose(
                    out=embT[:, k, :],
                    in_=emb_ext[:, :, k * 128 : (k + 1) * 128].rearrange(
                        "t b d -> (t b) d"
                    ),
                )

            qp_all = big.tile([128, T, D], BF16, tag="qpctx")
            kb_all = kballp.tile([128, T, D], BF16)
            scores = attnp.tile([128, T, R * H], F32)
            p2_all = attnp.tile([128, T, R * H], BF16)

            def emit_pair(t, j, r0, extras):
                """scores[t, r0-block] = per-head dot(Qp[t], Kb[j]) via
                bf16 mul + binary-tree halving over dh (dh-major layout:
                folding dh halves == folding contiguous column halves)."""
                prod = work.tile([128, D], BF16, tag="prod")
                nc.vector.tensor_mul(prod, qp_all[:, t, :], kb_all[:, j, :])
                nc.vector.tensor_add(prod[:, 0:512], prod[:, 0:512], prod[:, 512:1024])
                nc.vector.tensor_add(prod[:, 0:256], prod[:, 0:256], prod[:, 256:512])
                sc32 = smalls.tile([128, 128], F32, tag="sc32")
                nc.vector.tensor_add(sc32, prod[:, 0:128], prod[:, 128:256])
                nc.vector.tensor_add(sc32[:, 0:64], sc32[:, 0:64], sc32[:, 64:128])
                nc.vector.tensor_add(sc32[:, 0:32], sc32[:, 0:32], sc32[:, 32:64])
                dst = scores[:, t, r0 * H : (r0 + 1) * H]
                nc.vector.tensor_add(dst, sc32[:, 0:16], sc32[:, 16:32])
                for rx in extras:
                    nc.vector.tensor_copy(scores[:, t, rx * H : (rx + 1) * H], dst)

            def emit_softmax(t):
                s_t = scores[:, t, :]
                nc.vector.tensor_mul(s_t, s_t, rw_full[:, t, :])  # *= rel_w
                m = smalls.tile([128, H], F32, tag="m")
                nc.vector.reduce_max(
                    out=m, in_=s_t.rearrange("p (r h) -> p h r", h=H), axis=AX.X
                )
                e_t = p2_all[:, t, :]
                nc.vector.tensor_sub(
                    e_t.rearrange("p (r h) -> p r h", h=H),
                    s_t.rearrange("p (r h) -> p r h", h=H),
                    _bcast_free(m, R, 0),
                )
                # exp((s - m) / sqrt(DH)); the 1/8 rides ACT's free affine
                nc.scalar.activation(e_t, e_t, AF.Exp, scale=0.125)
                ssum = smalls.tile([128, H], F32, tag="ssum")
                nc.vector.reduce_sum(
                    out=ssum, in_=e_t.rearrange("p (r h) -> p h r", h=H), axis=AX.X
                )
                inv = smalls.tile([128, H], F32, tag="inv")
                nc.vector.reciprocal(inv, ssum)
                nc.vector.tensor_mul(
                    e_t.rearrange("p (r h) -> p r h", h=H),
                    e_t.rearrange("p (r h) -> p r h", h=H),
                    _bcast_free(inv, R, 0),
                )
                nc.vector.tensor_mul(e_t, e_t, rw_full[:, t, :])  # P = attn*rel_w
                for j, r0, extras in groups[t]:
                    for rx in extras:
                        nc.vector.tensor_add(
                            e_t[:, r0 * H : (r0 + 1) * H],
                            e_t[:, r0 * H : (r0 + 1) * H],
                            e_t[:, rx * H : (rx + 1) * H],
                        )

            # ---- phase 1: Q+K projections (dense PE stream) + trailing scores ----
            next_pair = 0
            for t in range(T):
                psq0 = psum.tile([128, 512], F32, tag="ps", name="ps")
                psq1 = psum.tile([128, 512], F32, tag="ps", name="ps")
                psk0 = psum.tile([128, 512], F32, tag="ps", name="ps")
                psk1 = psum.tile([128, 512], F32, tag="ps", name="ps")
                for k in range(KCH):
                    lhs = embT[:, k, t * BC : (t + 1) * BC]
                    fl = dict(start=(k == 0), stop=(k == KCH - 1))
                    nc.tensor.matmul(psq0, lhs, wq_t[:, k, 0:512], **fl)
                    nc.tensor.matmul(psq1, lhs, wq_t[:, k, 512:1024], **fl)
                    nc.tensor.matmul(psk0, lhs, wk_t[:, k, 0:512], **fl)
                    nc.tensor.matmul(psk1, lhs, wk_t[:, k, 512:1024], **fl)
                nc.scalar.copy(out=qp_all[:, t, 0:512], in_=psq0)
                nc.scalar.copy(out=qp_all[:, t, 512:1024], in_=psq1)
                nc.scalar.copy(out=kb_all[:, t, 0:512], in_=psk0)
                nc.scalar.copy(out=kb_all[:, t, 512:1024], in_=psk1)
                if use_bq:
                    nc.vector.tensor_add(qp_all[:, t, :], qp_all[:, t, :], bq_full)
                # emit score pairs that just became available
                while next_pair < len(pair_sched) and pair_sched[next_pair][0] <= t:
                    _, tp, jp, r0p, exp_ = pair_sched[next_pair]
                    emit_pair(tp, jp, r0p, exp_)
                    if last_pair_of_t[tp] == next_pair:
                        emit_softmax(tp)
                    next_pair += 1

            wv_t = wpool.tile([128, KCH, D], BF16, tag="w")  # reuses wq slot
            nc.gpsimd.dma_start(
                out=wv_t, in_=wv_ext.rearrange("(k p) o -> p k o", p=128)
            )
            wo_t = wpool.tile([128, KCH, D], BF16, tag="w")  # reuses wk slot
            nc.gpsimd.dma_start(
                out=wo_t, in_=wo_ext.rearrange("(k p) o -> p k o", p=128)
            )

            # ---- phase 2: V projections (recomputed) + ctx accumulation ----
            ctx_all = big.tile([128, T, D], BF16, tag="qpctx")  # reuses qp slot

            def emit_out_proj(t):
                """ctx[t] (SBUF, b-part) -> PE transpose -> Wo matmuls -> out DMA.
                Runs inline as soon as ctx[t] is complete, overlapping the
                DVE-bound ctx phase instead of a serial phase 3."""
                ctxT_ps = psumt.tile([128, KCH, 128], BF16, tag="tr", name="tr")
                for k in range(KCH):
                    nc.tensor.transpose(
                        out=ctxT_ps[:, k, :],
                        in_=ctx_all[:, t, k * 128 : (k + 1) * 128],
                        identity=ident,
                    )
                ctxT_sb = ctxTp.tile([128, KCH, 128], BF16)
                nc.scalar.copy(out=ctxT_sb, in_=ctxT_ps)
                o_t = outp.tile([128, D], F32)
                pso0 = psum.tile([128, 512], F32, tag="ps", name="ps")
                pso1 = psum.tile([128, 512], F32, tag="ps", name="ps")
                for k in range(KCH):
                    lhs = ctxT_sb[:, k, :]
                    fl = dict(start=(k == 0), stop=(k == KCH - 1))
                    nc.tensor.matmul(pso0, lhs, wo_t[:, k, 0:512], **fl)
                    nc.tensor.matmul(pso1, lhs, wo_t[:, k, 512:1024], **fl)
                nc.scalar.copy(out=o_t[:, 0:512], in_=pso0)
                nc.scalar.copy(out=o_t[:, 512:1024], in_=pso1)
                if use_bo:
                    nc.vector.tensor_add(o_t, o_t, bo_full)
                nc.sync.dma_start(out=out_ext[t], in_=o_t)

            # V pass: pure PE stream, all 16 tables SBUF-resident
            vb_all = vballp.tile([128, T, D], BF16)
            for j in range(T):
                psv0 = psum.tile([128, 512], F32, tag="ps", name="ps")
                psv1 = psum.tile([128, 512], F32, tag="ps", name="ps")
                for k in range(KCH):
                    lhs = embT[:, k, j * BC : (j + 1) * BC]
                    fl = dict(start=(k == 0), stop=(k == KCH - 1))
                    nc.tensor.matmul(psv0, lhs, wv_t[:, k, 0:512], **fl)
                    nc.tensor.matmul(psv1, lhs, wv_t[:, k, 512:1024], **fl)
                nc.scalar.copy(out=vb_all[:, j, 0:512], in_=psv0)
                nc.scalar.copy(out=vb_all[:, j, 512:1024], in_=psv1)

            # ctx accumulation t-major (no V dependency left), O-proj inline
            alt = 0
            for t in range(T):
                for gi, (j, r0, extras) in enumerate(groups[t]):
                    pb = _bcast_free(p2_all[:, t, r0 * H : (r0 + 1) * H], DH, 0)
                    vb3 = vb_all[:, j, :].rearrange("p (dh h) -> p dh h", h=H)
                    c3 = ctx_all[:, t, :].rearrange("p (dh h) -> p dh h", h=H)
                    if gi == 0:
                        nc.vector.tensor_mul(c3, vb3, pb)
                    else:
                        tmp = work.tile([128, D], BF16, tag="ctmp")
                        nc.vector.tensor_mul(
                            tmp.rearrange("p (dh h) -> p dh h", h=H), vb3, pb
                        )
                        eng = nc.vector if alt % 2 == 0 else nc.gpsimd
                        alt += 1
                        eng.tensor_add(ctx_all[:, t, :], ctx_all[:, t, :], tmp)
                emit_out_proj(t)

    return nc


_CACHE = {}


def _get_program(rel_idx, use_bq, use_bo):
    key = (rel_idx.tobytes(), use_bq, use_bo)
    if key not in _CACHE:
        nc = _build(rel_idx, use_bq, use_bo)
        nc.finalize()  # runs the bacc passes (reg alloc, wait lowering, ...)
        _CACHE[key] = nc
    return _CACHE[key]


def kernel(
    table_embs,
    rel_embs,
    rel_idx,
    Wq,
    bq,
    Wk,
    bk,
    Wv,
    bv,
    Wo,
    bo,
    w_rel,
    b_rel,
    _trace=False,
):
    table_embs = np.asarray(table_embs, dtype=np.float32)
    rel_embs = np.asarray(rel_embs, dtype=np.float32)
    rel_idx = np.asarray(rel_idx).astype(np.int64)
    Wq, Wk, Wv, Wo = (np.asarray(w, dtype=np.float32) for w in (Wq, Wk, Wv, Wo))
    bq, bk, bv, bo = (np.asarray(b, dtype=np.float32) for b in (bq, bk, bv, bo))
    w_rel = np.asarray(w_rel, dtype=np.float32)
    b_rel = np.asarray(b_rel, dtype=np.float32)

    # ---- host-side tiny prep ----
    rw = 1.0 / (1.0 + np.exp(-(rel_embs @ w_rel + b_rel[0])))  # [T, R] fp32
    rw_full = np.repeat(rw.astype(np.float32), H, axis=1)  # [T, R*H], col=r*16+h
    bf = ml_dtypes.bfloat16
    wq_p = np.ascontiguousarray(Wq.T[:, _PERM], dtype=bf)
    wk_p = np.ascontiguousarray(Wk.T[:, _PERM], dtype=bf)
    wv_p = np.ascontiguousarray(Wv.T[:, _PERM], dtype=bf)
    wo_p = np.ascontiguousarray(Wo.T[_PERM, :], dtype=bf)
    use_bq = bool(np.any(bq))
    bo_eff = Wo @ bv + bo
    use_bo = bool(np.any(bo_eff))
    bq_p = np.ascontiguousarray(bq[_PERM], dtype=np.float32)

    nc = _get_program(rel_idx, use_bq, use_bo)

    in_maps = []
    for c in range(NCORES):
        m = {
            "emb": np.ascontiguousarray(
                table_embs[:, c * BC : (c + 1) * BC, :], dtype=bf
            ),
            "wq": wq_p,
            "wk": wk_p,
            "wv": wv_p,
            "wo": wo_p,
            "rw": rw_full,
            "ident": np.eye(128, dtype=bf),
        }
        if use_bq:
            m["bqp"] = bq_p
        if use_bo:
            m["boe"] = bo_eff.astype(np.float32)
        in_maps.append(m)

    res = run_bass_kernel_spmd(nc, in_maps, list(range(NCORES)), trace=_trace)
    out = np.empty((T, B, D), dtype=np.float32)
    for c in range(NCORES):
        out[:, c * BC : (c + 1) * BC, :] = res.results[c]["out"]
    if _trace:
        kernel._last_results = res
    return out



# revision 19
# speedup vs baseline: 1.2749x; 1.0063x over previous
"""CrossTableAttention Trainium2 kernel (8-core SPMD, batch-sharded).

Math (per table t, row b):
  rel_w[t,r]   = sigmoid(rel_embs[t,r] . w_rel + b_rel)          (host, tiny)
  Qp[t]        = emb[t] @ Wq.T (+bq)                              [B, D]
  Kb[j]        = emb[j] @ Wk.T        (bias bk is softmax-invariant -> dropped)
  Vb[j]        = emb[j] @ Wv.T        (bias bv folded into output bias)
  score[t,b,h,r] = rel_w[t,r] * (Qp[t,b,h,:] . Kb[j_r,b,h,:]) / sqrt(DH)
  attn         = softmax_r(score);  P = attn * rel_w
  ctx[t,b]     = sum_r P[t,b,h,r] * Vb[j_r,b,h,:]
  out[t]       = ctx[t] @ Wo.T + (Wo @ bv + bo)

Key algebraic optimization: K/V projections are computed per *table* (16) instead
of per (table, relation) gather (128) - the relation weight is a scalar that
commutes with the linear projection.  5x fewer matmul FLOPs than the reference.

Device layout notes:
 - Activations live as [row(b) on partitions, feature on free] so the attention
   inner products reduce along the free axis (DVE).
 - q/k/v/ctx feature axis is permuted to dh-major (f = dh*16 + h) so the
   per-(b,h) attention-weight broadcast has a step-1 innermost AP dim
   (keeps DVE tensor_tensor in 2x bf16 mode).  Weight matrices are permuted
   host-side to produce/consume this layout directly.
 - scores/P layout per t: [128 b, 128 = r*16 + h] (h innermost).
 - Matmul operands need the contraction dim (d) on partitions; embT is produced
   by bf16 DMA-xbar transposing loads straight from the (host pre-cast) input.
"""

import sys

sys.path.insert(0, "/opt/trn_rl_repo")

import numpy as np
import ml_dtypes

import concourse.bass as bass
import concourse.bacc as bacc_mod
import concourse.mybir as mybir
import concourse.tile as tile
from concourse.bass_utils import run_bass_kernel_spmd

T, B, D, R, H = 16, 1024, 1024, 8, 16
DH = D // H  # 64
NCORES = 8
BC = B // NCORES  # 128 rows per core
KCH = D // 128  # 8 contraction chunks

F32 = mybir.dt.float32
BF16 = mybir.dt.bfloat16
AX = mybir.AxisListType
AF = mybir.ActivationFunctionType

# feature permutation: new f = dh*16 + h  <->  old o = h*64 + dh
_PERM = np.array([(f % H) * DH + f // H for f in range(D)], dtype=np.int64)


def _bcast_free(ap, n, pos):
    """Insert a [step=0, n] broadcast dim into an AP's free dims at `pos`
    (pos counts free dims, 0 = outermost free dim)."""
    new = list(ap.ap)
    new.insert(1 + pos, [0, n])
    return bass.AP(tensor=ap.tensor, offset=ap.offset, ap=new)


def _bcast_part(ap, n=128):
    """Broadcast a (DRAM) AP across n partitions by prepending a [0, n] dim."""
    return bass.AP(tensor=ap.tensor, offset=ap.offset, ap=[[0, n]] + list(ap.ap))


def _structure(rel_idx):
    """Host-side dedup of the (t, j) gather structure."""
    groups = []  # per t: list of (j, r0, [extra r's])
    for t in range(T):
        by_j = {}
        for r in range(R):
            by_j.setdefault(int(rel_idx[t, r]), []).append(r)
        groups.append([(j, rs[0], rs[1:]) for j, rs in sorted(by_j.items())])
    pairs_by_j = {j: [] for j in range(T)}
    for t in range(T):
        for j, r0, extras in groups[t]:
            pairs_by_j[j].append((t, r0))
    # first j (in ascending j processing order) touching each t
    first_j = {}
    for j in range(T):
        for t, _ in pairs_by_j[j]:
            if t not in first_j:
                first_j[t] = j
    # last j touching each t (to schedule ctx output DMA)
    last_j = {}
    for j in range(T):
        for t, _ in pairs_by_j[j]:
            last_j[t] = j
    return groups, pairs_by_j, first_j, last_j


def _build(rel_idx, use_bq, use_bo):
    """Build the SPMD bass program (identical on all cores).

    v2 structure (from v1 trace analysis):
     - Q+K projections interleaved per t (shared stationary embT chunk ->
       LDWEIGHTS amortized over 4 matmuls), Kb fully SBUF-resident so the
       scores phase never back-pressures the PE.
     - score pairs emitted in availability order (sorted by max(t, j)) so the
       DVE trails the projection stream without stalls.
     - per-head dot products via a binary tree of tensor_tensor adds (bf16 2x
       mode for the large levels, fp32 tail) instead of 1x tensor_reduce.
     - V projections recomputed in the ctx phase (keeps PE busy there);
       ctx accumulation adds alternate DVE / GPSIMD.
    """
    groups, pairs_by_j, first_j, last_j = _structure(rel_idx)

    # (t, j, r0, extras) sorted by when both Qp[t] and Kb[j] become available
    pair_sched = []
    for t in range(T):
        for j, r0, extras in groups[t]:
            pair_sched.append((max(t, j), t, j, r0, extras))
    pair_sched.sort(key=lambda x: (x[0], x[1]))
    last_pair_of_t = {}
    for i, (_, t, j, r0, ex) in enumerate(pair_sched):
        last_pair_of_t[t] = i

    nc = bacc_mod.Bacc(None, target_bir_lowering=False, debug=False)
    emb_ext = nc.dram_tensor("emb", [T, BC, D], BF16, kind="ExternalInput")
    wq_ext = nc.dram_tensor("wq", [D, D], BF16, kind="ExternalInput")
    wk_ext = nc.dram_tensor("wk", [D, D], BF16, kind="ExternalInput")
    wv_ext = nc.dram_tensor("wv", [D, D], BF16, kind="ExternalInput")
    wo_ext = nc.dram_tensor("wo", [D, D], BF16, kind="ExternalInput")
    rw_ext = nc.dram_tensor("rw", [T, R * H], F32, kind="ExternalInput")
    id_ext = nc.dram_tensor("ident", [128, 128], BF16, kind="ExternalInput")
    if use_bq:
        bq_ext = nc.dram_tensor("bqp", [D], F32, kind="ExternalInput")
    if use_bo:
        bo_ext = nc.dram_tensor("boe", [D], F32, kind="ExternalInput")
    out_ext = nc.dram_tensor("out", [T, BC, D], F32, kind="ExternalOutput")

    with tile.TileContext(nc) as tc:
        with (
            tc.tile_pool(name="consts", bufs=1) as consts,
            tc.tile_pool(name="wpool", bufs=2) as wpool,
            tc.tile_pool(name="big", bufs=1) as big,
            tc.tile_pool(name="embp", bufs=1) as embp,
            tc.tile_pool(name="kball", bufs=1) as kballp,
            tc.tile_pool(name="vball", bufs=1) as vballp,
            tc.tile_pool(name="attn", bufs=1) as attnp,
            tc.tile_pool(name="work", bufs=3) as work,
            tc.tile_pool(name="ctxsb", bufs=2) as ctxsbp,
            tc.tile_pool(name="smalls", bufs=3) as smalls,
            tc.tile_pool(name="outp", bufs=1) as outp,
            tc.tile_pool(name="ctxT", bufs=2) as ctxTp,
            tc.tile_pool(name="psum", bufs=4, space="PSUM") as psum,
            tc.tile_pool(name="psumt", bufs=1, space="PSUM") as psumt,
            tc.tile_pool(name="psumc", bufs=1, space="PSUM") as psumc,
        ):
            # ---- constant + weight loads ----
            ident = consts.tile([128, 128], BF16)
            nc.gpsimd.dma_start(out=ident, in_=id_ext[:])
            rw_full = consts.tile([128, T, R * H], F32)
            nc.gpsimd.dma_start(out=rw_full, in_=_bcast_part(rw_ext[:]))
            if use_bq:
                bq_full = consts.tile([128, D], F32)
                nc.gpsimd.dma_start(out=bq_full, in_=_bcast_part(bq_ext[:]))
            if use_bo:
                bo_full = consts.tile([128, D], F32)
                nc.gpsimd.dma_start(out=bo_full, in_=_bcast_part(bo_ext[:]))

            # weights split per-k so matmul k=0 can start after ~1/8 of the load
            wq_t = wpool.tile([128, KCH, D], BF16, tag="w")
            wk_t = wpool.tile([128, KCH, D], BF16, tag="w")
            wq_r = wq_ext.rearrange("(k p) o -> p k o", p=128)
            wk_r = wk_ext.rearrange("(k p) o -> p k o", p=128)
            for k in range(KCH):
                nc.gpsimd.dma_start(out=wq_t[:, k, :], in_=wq_r[:, k, :])
                nc.gpsimd.dma_start(out=wk_t[:, k, :], in_=wk_r[:, k, :])

            # embT[p, k, t*BC+b] = emb[t, b, k*128+p]  (bf16 xbar transposes)
            embT = embp.tile([128, KCH, T * BC], BF16)
            for k in range(KCH):
                nc.scalar.dma_start_transpose(
                    out=embT[:, k, :],
                    in_=emb_ext[:, :, k * 128 : (k + 1) * 128].rearrange(
                        "t b d -> (t b) d"
                    ),
                )

            qp_all = big.tile([128, T, D], BF16, tag="qpctx")
            kb_all = kballp.tile([128, T, D], BF16)
            scores = attnp.tile([128, T, R * H], F32)
            p2_all = attnp.tile([128, T, R * H], BF16)

            def emit_pair(t, j, r0, extras):
                """scores[t, r0-block] = per-head dot(Qp[t], Kb[j]) via
                bf16 mul + binary-tree halving over dh (dh-major layout:
                folding dh halves == folding contiguous column halves)."""
                prod = work.tile([128, D], BF16, tag="prod")
                nc.vector.tensor_mul(prod, qp_all[:, t, :], kb_all[:, j, :])
                nc.vector.tensor_add(prod[:, 0:512], prod[:, 0:512], prod[:, 512:1024])
                nc.vector.tensor_add(prod[:, 0:256], prod[:, 0:256], prod[:, 256:512])
                sc32 = smalls.tile([128, 128], F32, tag="sc32")
                nc.vector.tensor_add(sc32, prod[:, 0:128], prod[:, 128:256])
                nc.vector.tensor_add(sc32[:, 0:64], sc32[:, 0:64], sc32[:, 64:128])
                nc.vector.tensor_add(sc32[:, 0:32], sc32[:, 0:32], sc32[:, 32:64])
                dst = scores[:, t, r0 * H : (r0 + 1) * H]
                nc.vector.tensor_add(dst, sc32[:, 0:16], sc32[:, 16:32])
                for rx in extras:
                    nc.vector.tensor_copy(scores[:, t, rx * H : (rx + 1) * H], dst)

            def emit_softmax(t):
                s_t = scores[:, t, :]
                nc.vector.tensor_mul(s_t, s_t, rw_full[:, t, :])  # *= rel_w
                m = smalls.tile([128, H], F32, tag="m")
                nc.vector.reduce_max(
                    out=m, in_=s_t.rearrange("p (r h) -> p h r", h=H), axis=AX.X
                )
                e_t = p2_all[:, t, :]
                nc.vector.tensor_sub(
                    e_t.rearrange("p (r h) -> p r h", h=H),
                    s_t.rearrange("p (r h) -> p r h", h=H),
                    _bcast_free(m, R, 0),
                )
                # exp((s - m) / sqrt(DH)); the 1/8 rides ACT's free affine
                nc.scalar.activation(e_t, e_t, AF.Exp, scale=0.125)
                ssum = smalls.tile([128, H], F32, tag="ssum")
                nc.vector.reduce_sum(
                    out=ssum, in_=e_t.rearrange("p (r h) -> p h r", h=H), axis=AX.X
                )
                inv = smalls.tile([128, H], F32, tag="inv")
                nc.vector.reciprocal(inv, ssum)
                nc.vector.tensor_mul(
                    e_t.rearrange("p (r h) -> p r h", h=H),
                    e_t.rearrange("p (r h) -> p r h", h=H),
                    _bcast_free(inv, R, 0),
                )
                nc.vector.tensor_mul(e_t, e_t, rw_full[:, t, :])  # P = attn*rel_w
                for j, r0, extras in groups[t]:
                    for rx in extras:
                        nc.vector.tensor_add(
                            e_t[:, r0 * H : (r0 + 1) * H],
                            e_t[:, r0 * H : (r0 + 1) * H],
                            e_t[:, rx * H : (rx + 1) * H],
                        )

            # ---- phase 1: Q+K projections (dense PE stream) + trailing scores ----
            next_pair = 0
            for t in range(T):
                psq0 = psum.tile([128, 512], F32, tag="ps", name="ps")
                psq1 = psum.tile([128, 512], F32, tag="ps", name="ps")
                psk0 = psum.tile([128, 512], F32, tag="ps", name="ps")
                psk1 = psum.tile([128, 512], F32, tag="ps", name="ps")
                for k in range(KCH):
                    lhs = embT[:, k, t * BC : (t + 1) * BC]
                    fl = dict(start=(k == 0), stop=(k == KCH - 1))
                    nc.tensor.matmul(psq0, lhs, wq_t[:, k, 0:512], **fl)
                    nc.tensor.matmul(psq1, lhs, wq_t[:, k, 512:1024], **fl)
                    nc.tensor.matmul(psk0, lhs, wk_t[:, k, 0:512], **fl)
                    nc.tensor.matmul(psk1, lhs, wk_t[:, k, 512:1024], **fl)
                nc.scalar.copy(out=qp_all[:, t, 0:512], in_=psq0)
                nc.scalar.copy(out=qp_all[:, t, 512:1024], in_=psq1)
                nc.scalar.copy(out=kb_all[:, t, 0:512], in_=psk0)
                nc.scalar.copy(out=kb_all[:, t, 512:1024], in_=psk1)
                if use_bq:
                    nc.vector.tensor_add(qp_all[:, t, :], qp_all[:, t, :], bq_full)
                # emit score pairs that just became available
                while next_pair < len(pair_sched) and pair_sched[next_pair][0] <= t:
                    _, tp, jp, r0p, exp_ = pair_sched[next_pair]
                    emit_pair(tp, jp, r0p, exp_)
                    if last_pair_of_t[tp] == next_pair:
                        emit_softmax(tp)
                    next_pair += 1

            wv_t = wpool.tile([128, KCH, D], BF16, tag="w")  # reuses wq slot
            nc.gpsimd.dma_start(
                out=wv_t, in_=wv_ext.rearrange("(k p) o -> p k o", p=128)
            )
            wo_t = wpool.tile([128, KCH, D], BF16, tag="w")  # reuses wk slot
            nc.gpsimd.dma_start(
                out=wo_t, in_=wo_ext.rearrange("(k p) o -> p k o", p=128)
            )

            # ---- phase 2: V projections (recomputed) + ctx accumulation ----

            def emit_out_proj(t, ctx_sb):
                """ctx[t] (SBUF, b-part) -> PE transpose -> Wo matmuls -> out DMA.
                Runs inline as soon as ctx[t] is complete, overlapping the
                DVE-bound ctx phase instead of a serial phase 3."""
                ctxT_ps = psumt.tile([128, KCH, 128], BF16, tag="tr", name="tr")
                for k in range(KCH):
                    nc.tensor.transpose(
                        out=ctxT_ps[:, k, :],
                        in_=ctx_sb[:, k * 128 : (k + 1) * 128],
                        identity=ident,
                    )
                ctxT_sb = ctxTp.tile([128, KCH, 128], BF16)
                nc.scalar.copy(out=ctxT_sb, in_=ctxT_ps)
                o_t = outp.tile([128, D], F32)
                pso0 = psum.tile([128, 512], F32, tag="ps", name="ps")
                pso1 = psum.tile([128, 512], F32, tag="ps", name="ps")
                for k in range(KCH):
                    lhs = ctxT_sb[:, k, :]
                    fl = dict(start=(k == 0), stop=(k == KCH - 1))
                    nc.tensor.matmul(pso0, lhs, wo_t[:, k, 0:512], **fl)
                    nc.tensor.matmul(pso1, lhs, wo_t[:, k, 512:1024], **fl)
                nc.scalar.copy(out=o_t[:, 0:512], in_=pso0)
                nc.scalar.copy(out=o_t[:, 512:1024], in_=pso1)
                if use_bo:
                    nc.vector.tensor_add(o_t, o_t, bo_full)
                nc.sync.dma_start(out=out_ext[t], in_=o_t)

            # V pass: pure PE stream, all 16 tables SBUF-resident
            vb_all = vballp.tile([128, T, D], BF16)
            for j in range(T):
                psv0 = psum.tile([128, 512], F32, tag="ps", name="ps")
                psv1 = psum.tile([128, 512], F32, tag="ps", name="ps")
                for k in range(KCH):
                    lhs = embT[:, k, j * BC : (j + 1) * BC]
                    fl = dict(start=(k == 0), stop=(k == KCH - 1))
                    nc.tensor.matmul(psv0, lhs, wv_t[:, k, 0:512], **fl)
                    nc.tensor.matmul(psv1, lhs, wv_t[:, k, 512:1024], **fl)
                nc.scalar.copy(out=vb_all[:, j, 0:512], in_=psv0)
                nc.scalar.copy(out=vb_all[:, j, 512:1024], in_=psv1)

            # ctx accumulation t-major: per-pair weighted products (DVE/GpSimd
            # muls, 2:1 split) accumulate in PSUM via identity matmuls on the
            # PE (which has slack here), eliminating the DVE/GpSimd add chain.
            alt = 0
            for t in range(T):
                ctx_ps = psumc.tile([128, D], F32, tag="cps", name="cps")
                n = len(groups[t])
                for gi, (j, r0, extras) in enumerate(groups[t]):
                    pb = _bcast_free(p2_all[:, t, r0 * H : (r0 + 1) * H], DH, 0)
                    vb3 = vb_all[:, j, :].rearrange("p (dh h) -> p dh h", h=H)
                    tmp = work.tile([128, D], BF16, tag="ctmp")
                    eng = nc.gpsimd if alt % 3 == 2 else nc.vector
                    alt += 1
                    eng.tensor_mul(
                        tmp.rearrange("p (dh h) -> p dh h", h=H), vb3, pb
                    )
                    fl = dict(start=(gi == 0), stop=(gi == n - 1))
                    nc.tensor.matmul(ctx_ps[:, 0:512], ident, tmp[:, 0:512], **fl)
                    nc.tensor.matmul(
                        ctx_ps[:, 512:1024], ident, tmp[:, 512:1024], **fl
                    )
                ctx_sb = ctxsbp.tile([128, D], BF16)
                nc.scalar.copy(out=ctx_sb[:, 0:512], in_=ctx_ps[:, 0:512])
                nc.scalar.copy(out=ctx_sb[:, 512:1024], in_=ctx_ps[:, 512:1024])
                emit_out_proj(t, ctx_sb)

    return nc


_CACHE = {}


def _get_program(rel_idx, use_bq, use_bo):
    key = (rel_idx.tobytes(), use_bq, use_bo)
    if key not in _CACHE:
        nc = _build(rel_idx, use_bq, use_bo)
        nc.finalize()  # runs the bacc passes (reg alloc, wait lowering, ...)
        _CACHE[key] = nc
    return _CACHE[key]


def kernel(
    table_embs,
    rel_embs,
    rel_idx,
    Wq,
    bq,
    Wk,
    bk,
    Wv,
    bv,
    Wo,
    bo,
    w_rel,
    b_rel,
    _trace=False,
):
    table_embs = np.asarray(table_embs, dtype=np.float32)
    rel_embs = np.asarray(rel_embs, dtype=np.float32)
    rel_idx = np.asarray(rel_idx).astype(np.int64)
    Wq, Wk, Wv, Wo = (np.asarray(w, dtype=np.float32) for w in (Wq, Wk, Wv, Wo))
    bq, bk, bv, bo = (np.asarray(b, dtype=np.float32) for b in (bq, bk, bv, bo))
    w_rel = np.asarray(w_rel, dtype=np.float32)
    b_rel = np.asarray(b_rel, dtype=np.float32)

    # ---- host-side tiny prep ----
    rw = 1.0 / (1.0 + np.exp(-(rel_embs @ w_rel + b_rel[0])))  # [T, R] fp32
    rw_full = np.repeat(rw.astype(np.float32), H, axis=1)  # [T, R*H], col=r*16+h
    bf = ml_dtypes.bfloat16
    wq_p = np.ascontiguousarray(Wq.T[:, _PERM], dtype=bf)
    wk_p = np.ascontiguousarray(Wk.T[:, _PERM], dtype=bf)
    wv_p = np.ascontiguousarray(Wv.T[:, _PERM], dtype=bf)
    wo_p = np.ascontiguousarray(Wo.T[_PERM, :], dtype=bf)
    use_bq = bool(np.any(bq))
    bo_eff = Wo @ bv + bo
    use_bo = bool(np.any(bo_eff))
    bq_p = np.ascontiguousarray(bq[_PERM], dtype=np.float32)

    nc = _get_program(rel_idx, use_bq, use_bo)

    in_maps = []
    for c in range(NCORES):
        m = {
            "emb": np.ascontiguousarray(
                table_embs[:, c * BC : (c + 1) * BC, :], dtype=bf
            ),
            "wq": wq_p,
            "wk": wk_p,
            "wv": wv_p,
            "wo": wo_p,
            "rw": rw_full,
            "ident": np.eye(128, dtype=bf),
        }
        if use_bq:
            m["bqp"] = bq_p
        if use_bo:
            m["boe"] = bo_eff.astype(np.float32)
        in_maps.append(m)

    res = run_bass_kernel_spmd(nc, in_maps, list(range(NCORES)), trace=_trace)
    out = np.empty((T, B, D), dtype=np.float32)
    for c in range(NCORES):
        out[:, c * BC : (c + 1) * BC, :] = res.results[c]["out"]
    if _trace:
        kernel._last_results = res
    return out



# revision 29
# speedup vs baseline: 1.3494x; 1.0584x over previous
"""CrossTableAttention Trainium2 kernel (8-core SPMD, batch-sharded).

Math (per table t, row b):
  rel_w[t,r]   = sigmoid(rel_embs[t,r] . w_rel + b_rel)          (host, tiny)
  Qp[t]        = emb[t] @ Wq.T (+bq)                              [B, D]
  Kb[j]        = emb[j] @ Wk.T        (bias bk is softmax-invariant -> dropped)
  Vb[j]        = emb[j] @ Wv.T        (bias bv folded into output bias)
  score[t,b,h,r] = rel_w[t,r] * (Qp[t,b,h,:] . Kb[j_r,b,h,:]) / sqrt(DH)
  attn         = softmax_r(score);  P = attn * rel_w
  ctx[t,b]     = sum_r P[t,b,h,r] * Vb[j_r,b,h,:]
  out[t]       = ctx[t] @ Wo.T + (Wo @ bv + bo)

Key algebraic optimization: K/V projections are computed per *table* (16) instead
of per (table, relation) gather (128) - the relation weight is a scalar that
commutes with the linear projection.  5x fewer matmul FLOPs than the reference.

Device layout notes:
 - Activations live as [row(b) on partitions, feature on free] so the attention
   inner products reduce along the free axis (DVE).
 - q/k/v/ctx feature axis is permuted to dh-major (f = dh*16 + h) so the
   per-(b,h) attention-weight broadcast has a step-1 innermost AP dim
   (keeps DVE tensor_tensor in 2x bf16 mode).  Weight matrices are permuted
   host-side to produce/consume this layout directly.
 - scores/P layout per t: [128 b, 128 = r*16 + h] (h innermost).
 - Matmul operands need the contraction dim (d) on partitions; embT is produced
   by bf16 DMA-xbar transposing loads straight from the (host pre-cast) input.
"""

import sys

sys.path.insert(0, "/opt/trn_rl_repo")

import numpy as np
import ml_dtypes

import concourse.bass as bass
import concourse.bacc as bacc_mod
import concourse.mybir as mybir
import concourse.tile as tile
from concourse.bass_utils import run_bass_kernel_spmd

T, B, D, R, H = 16, 1024, 1024, 8, 16
DH = D // H  # 64
NCORES = 8
BC = B // NCORES  # 128 rows per core
KCH = D // 128  # 8 contraction chunks

F32 = mybir.dt.float32
BF16 = mybir.dt.bfloat16
AX = mybir.AxisListType
AF = mybir.ActivationFunctionType

# feature permutation: new f = dh*16 + h  <->  old o = h*64 + dh
_PERM = np.array([(f % H) * DH + f // H for f in range(D)], dtype=np.int64)


def _bcast_free(ap, n, pos):
    """Insert a [step=0, n] broadcast dim into an AP's free dims at `pos`
    (pos counts free dims, 0 = outermost free dim)."""
    new = list(ap.ap)
    new.insert(1 + pos, [0, n])
    return bass.AP(tensor=ap.tensor, offset=ap.offset, ap=new)


def _bcast_part(ap, n=128):
    """Broadcast a (DRAM) AP across n partitions by prepending a [0, n] dim."""
    return bass.AP(tensor=ap.tensor, offset=ap.offset, ap=[[0, n]] + list(ap.ap))


def _structure(rel_idx):
    """Host-side dedup of the (t, j) gather structure."""
    groups = []  # per t: list of (j, r0, [extra r's])
    for t in range(T):
        by_j = {}
        for r in range(R):
            by_j.setdefault(int(rel_idx[t, r]), []).append(r)
        groups.append([(j, rs[0], rs[1:]) for j, rs in sorted(by_j.items())])
    pairs_by_j = {j: [] for j in range(T)}
    for t in range(T):
        for j, r0, extras in groups[t]:
            pairs_by_j[j].append((t, r0))
    # first j (in ascending j processing order) touching each t
    first_j = {}
    for j in range(T):
        for t, _ in pairs_by_j[j]:
            if t not in first_j:
                first_j[t] = j
    # last j touching each t (to schedule ctx output DMA)
    last_j = {}
    for j in range(T):
        for t, _ in pairs_by_j[j]:
            last_j[t] = j
    return groups, pairs_by_j, first_j, last_j


def _build(rel_idx, use_bq, use_bo):
    """Build the SPMD bass program (identical on all cores).

    v2 structure (from v1 trace analysis):
     - Q+K projections interleaved per t (shared stationary embT chunk ->
       LDWEIGHTS amortized over 4 matmuls), Kb fully SBUF-resident so the
       scores phase never back-pressures the PE.
     - score pairs emitted in availability order (sorted by max(t, j)) so the
       DVE trails the projection stream without stalls.
     - per-head dot products via a binary tree of tensor_tensor adds (bf16 2x
       mode for the large levels, fp32 tail) instead of 1x tensor_reduce.
     - V projections recomputed in the ctx phase (keeps PE busy there);
       ctx accumulation adds alternate DVE / GPSIMD.
    """
    groups, pairs_by_j, first_j, last_j = _structure(rel_idx)

    # (t, j, r0, extras) sorted by when both Qp[t] and Kb[j] become available
    pair_sched = []
    for t in range(T):
        for j, r0, extras in groups[t]:
            pair_sched.append((max(t, j), t, j, r0, extras))
    pair_sched.sort(key=lambda x: (x[0], x[1]))
    last_pair_of_t = {}
    for i, (_, t, j, r0, ex) in enumerate(pair_sched):
        last_pair_of_t[t] = i

    nc = bacc_mod.Bacc(None, target_bir_lowering=False, debug=False)
    emb_ext = nc.dram_tensor("emb", [T, BC, D], BF16, kind="ExternalInput")
    wq_ext = nc.dram_tensor("wq", [D, D], BF16, kind="ExternalInput")
    wk_ext = nc.dram_tensor("wk", [D, D], BF16, kind="ExternalInput")
    wv_ext = nc.dram_tensor("wv", [D, D], BF16, kind="ExternalInput")
    wo_ext = nc.dram_tensor("wo", [D, D], BF16, kind="ExternalInput")
    rw_ext = nc.dram_tensor("rw", [T, R * H], BF16, kind="ExternalInput")
    id_ext = nc.dram_tensor("ident", [128, 128], BF16, kind="ExternalInput")
    if use_bq:
        bq_ext = nc.dram_tensor("bqp", [D], F32, kind="ExternalInput")
    if use_bo:
        bo_ext = nc.dram_tensor("boe", [D], F32, kind="ExternalInput")
    out_ext = nc.dram_tensor("out", [T, BC, D], BF16, kind="ExternalOutput")

    with tile.TileContext(nc) as tc:
        with (
            tc.tile_pool(name="consts", bufs=1) as consts,
            tc.tile_pool(name="wpool", bufs=2) as wpool,
            tc.tile_pool(name="big", bufs=1) as big,
            tc.tile_pool(name="embp", bufs=1) as embp,
            tc.tile_pool(name="kball", bufs=1) as kballp,
            tc.tile_pool(name="vball", bufs=1) as vballp,
            tc.tile_pool(name="attn", bufs=1) as attnp,
            tc.tile_pool(name="work", bufs=3) as work,
            tc.tile_pool(name="ctxsb", bufs=2) as ctxsbp,
            tc.tile_pool(name="smalls", bufs=3) as smalls,
            tc.tile_pool(name="outp", bufs=2) as outp,
            tc.tile_pool(name="ctmpp", bufs=5) as ctmpp,
            tc.tile_pool(name="ctxT", bufs=2) as ctxTp,
            tc.tile_pool(name="psum", bufs=4, space="PSUM") as psum,
            tc.tile_pool(name="psumt", bufs=1, space="PSUM") as psumt,
            tc.tile_pool(name="psumc", bufs=1, space="PSUM") as psumc,
        ):
            # ---- constant + weight loads ----
            ident = consts.tile([128, 128], BF16)
            nc.gpsimd.dma_start(out=ident, in_=id_ext[:])
            rw_full = consts.tile([128, T, R * H], BF16)
            nc.gpsimd.dma_start(out=rw_full, in_=_bcast_part(rw_ext[:]))
            if use_bq:
                bq_full = consts.tile([128, D], F32)
                nc.gpsimd.dma_start(out=bq_full, in_=_bcast_part(bq_ext[:]))
            if use_bo:
                bo_full = consts.tile([128, D], F32)
                nc.gpsimd.dma_start(out=bo_full, in_=_bcast_part(bo_ext[:]))

            wq_t = wpool.tile([128, KCH, D], BF16, tag="w")
            nc.gpsimd.dma_start(
                out=wq_t, in_=wq_ext.rearrange("(k p) o -> p k o", p=128)
            )
            wk_t = wpool.tile([128, KCH, D], BF16, tag="w")
            nc.gpsimd.dma_start(
                out=wk_t, in_=wk_ext.rearrange("(k p) o -> p k o", p=128)
            )

            # embT[p, k, t*BC+b] = emb[t, b, k*128+p]  (bf16 xbar transposes)
            embT = embp.tile([128, KCH, T * BC], BF16)
            for k in range(KCH):
                nc.scalar.dma_start_transpose(
                    out=embT[:, k, :],
                    in_=emb_ext[:, :, k * 128 : (k + 1) * 128].rearrange(
                        "t b d -> (t b) d"
                    ),
                )

            qp_all = big.tile([128, T, D], BF16, tag="qpctx")
            kb_all = kballp.tile([128, T, D], BF16)
            scores = attnp.tile([128, T, R * H], F32)
            p2_all = attnp.tile([128, T, R * H], BF16)

            def emit_pair(t, j, r0, extras):
                """scores[t, r0-block] = per-head dot(Qp[t], Kb[j]) via
                bf16 mul + binary-tree halving over dh (dh-major layout:
                folding dh halves == folding contiguous column halves)."""
                prod = work.tile([128, D], BF16, tag="prod")
                nc.vector.tensor_mul(prod, qp_all[:, t, :], kb_all[:, j, :])
                nc.vector.tensor_add(prod[:, 0:512], prod[:, 0:512], prod[:, 512:1024])
                nc.vector.tensor_add(prod[:, 0:256], prod[:, 0:256], prod[:, 256:512])
                sc32 = smalls.tile([128, 128], F32, tag="sc32")
                nc.vector.tensor_add(sc32, prod[:, 0:128], prod[:, 128:256])
                nc.vector.tensor_add(sc32[:, 0:64], sc32[:, 0:64], sc32[:, 64:128])
                nc.vector.tensor_add(sc32[:, 0:32], sc32[:, 0:32], sc32[:, 32:64])
                dst = scores[:, t, r0 * H : (r0 + 1) * H]
                nc.vector.tensor_add(dst, sc32[:, 0:16], sc32[:, 16:32])
                for rx in extras:
                    nc.vector.tensor_copy(scores[:, t, rx * H : (rx + 1) * H], dst)

            def emit_softmax(t):
                s_t = scores[:, t, :]
                nc.vector.tensor_mul(s_t, s_t, rw_full[:, t, :])  # *= rel_w
                m = smalls.tile([128, H], F32, tag="m")
                nc.vector.reduce_max(
                    out=m, in_=s_t.rearrange("p (r h) -> p h r", h=H), axis=AX.X
                )
                e_t = p2_all[:, t, :]
                nc.vector.tensor_sub(
                    e_t.rearrange("p (r h) -> p r h", h=H),
                    s_t.rearrange("p (r h) -> p r h", h=H),
                    _bcast_free(m, R, 0),
                )
                # exp((s - m) / sqrt(DH)); the 1/8 rides ACT's free affine
                nc.scalar.activation(e_t, e_t, AF.Exp, scale=0.125)
                ssum = smalls.tile([128, H], F32, tag="ssum")
                nc.vector.reduce_sum(
                    out=ssum, in_=e_t.rearrange("p (r h) -> p h r", h=H), axis=AX.X
                )
                inv = smalls.tile([128, H], F32, tag="inv")
                nc.vector.reciprocal(inv, ssum)
                nc.vector.tensor_mul(
                    e_t.rearrange("p (r h) -> p r h", h=H),
                    e_t.rearrange("p (r h) -> p r h", h=H),
                    _bcast_free(inv, R, 0),
                )
                nc.vector.tensor_mul(e_t, e_t, rw_full[:, t, :])  # P = attn*rel_w
                for j, r0, extras in groups[t]:
                    for rx in extras:
                        nc.vector.tensor_add(
                            e_t[:, r0 * H : (r0 + 1) * H],
                            e_t[:, r0 * H : (r0 + 1) * H],
                            e_t[:, rx * H : (rx + 1) * H],
                        )

            # ---- phase 1: Q+K projections (dense PE stream) + trailing scores ----
            next_pair = 0
            for t in range(T):
                psq0 = psum.tile([128, 512], F32, tag="ps", name="ps")
                psq1 = psum.tile([128, 512], F32, tag="ps", name="ps")
                psk0 = psum.tile([128, 512], F32, tag="ps", name="ps")
                psk1 = psum.tile([128, 512], F32, tag="ps", name="ps")
                for k in range(KCH):
                    lhs = embT[:, k, t * BC : (t + 1) * BC]
                    fl = dict(start=(k == 0), stop=(k == KCH - 1))
                    nc.tensor.matmul(psq0, lhs, wq_t[:, k, 0:512], **fl)
                    nc.tensor.matmul(psq1, lhs, wq_t[:, k, 512:1024], **fl)
                    nc.tensor.matmul(psk0, lhs, wk_t[:, k, 0:512], **fl)
                    nc.tensor.matmul(psk1, lhs, wk_t[:, k, 512:1024], **fl)
                nc.scalar.copy(out=qp_all[:, t, 0:512], in_=psq0)
                nc.scalar.copy(out=qp_all[:, t, 512:1024], in_=psq1)
                nc.scalar.copy(out=kb_all[:, t, 0:512], in_=psk0)
                nc.scalar.copy(out=kb_all[:, t, 512:1024], in_=psk1)
                if use_bq:
                    nc.vector.tensor_add(qp_all[:, t, :], qp_all[:, t, :], bq_full)
                # emit score pairs that just became available
                while next_pair < len(pair_sched) and pair_sched[next_pair][0] <= t:
                    _, tp, jp, r0p, exp_ = pair_sched[next_pair]
                    emit_pair(tp, jp, r0p, exp_)
                    if last_pair_of_t[tp] == next_pair:
                        emit_softmax(tp)
                    next_pair += 1

            wv_t = wpool.tile([128, KCH, D], BF16, tag="w")  # reuses wq slot
            nc.gpsimd.dma_start(
                out=wv_t, in_=wv_ext.rearrange("(k p) o -> p k o", p=128)
            )
            wo_t = wpool.tile([128, KCH, D], BF16, tag="w")  # reuses wk slot
            nc.gpsimd.dma_start(
                out=wo_t, in_=wo_ext.rearrange("(k p) o -> p k o", p=128)
            )

            # ---- phase 2: V projections (recomputed) + ctx accumulation ----

            def emit_out_proj(t, ctx_sb):
                """ctx[t] (SBUF, b-part) -> PE transpose -> Wo matmuls -> out DMA.
                Runs inline as soon as ctx[t] is complete, overlapping the
                DVE-bound ctx phase instead of a serial phase 3."""
                ctxT_ps = psumt.tile([128, KCH, 128], BF16, tag="tr", name="tr")
                for k in range(KCH):
                    nc.tensor.transpose(
                        out=ctxT_ps[:, k, :],
                        in_=ctx_sb[:, k * 128 : (k + 1) * 128],
                        identity=ident,
                    )
                ctxT_sb = ctxTp.tile([128, KCH, 128], BF16)
                nc.scalar.copy(out=ctxT_sb, in_=ctxT_ps)
                o_t = outp.tile([128, D], BF16)
                pso0 = psum.tile([128, 512], F32, tag="ps", name="ps")
                pso1 = psum.tile([128, 512], F32, tag="ps", name="ps")
                for k in range(KCH):
                    lhs = ctxT_sb[:, k, :]
                    fl = dict(start=(k == 0), stop=(k == KCH - 1))
                    nc.tensor.matmul(pso0, lhs, wo_t[:, k, 0:512], **fl)
                    nc.tensor.matmul(pso1, lhs, wo_t[:, k, 512:1024], **fl)
                nc.scalar.copy(out=o_t[:, 0:512], in_=pso0)
                nc.scalar.copy(out=o_t[:, 512:1024], in_=pso1)
                if use_bo:
                    nc.vector.tensor_add(o_t, o_t, bo_full)
                nc.sync.dma_start(out=out_ext[t], in_=o_t)

            # V pass: pure PE stream, all 16 tables SBUF-resident
            vb_all = vballp.tile([128, T, D], BF16)
            for j in range(T):
                psv0 = psum.tile([128, 512], F32, tag="ps", name="ps")
                psv1 = psum.tile([128, 512], F32, tag="ps", name="ps")
                for k in range(KCH):
                    lhs = embT[:, k, j * BC : (j + 1) * BC]
                    fl = dict(start=(k == 0), stop=(k == KCH - 1))
                    nc.tensor.matmul(psv0, lhs, wv_t[:, k, 0:512], **fl)
                    nc.tensor.matmul(psv1, lhs, wv_t[:, k, 512:1024], **fl)
                nc.scalar.copy(out=vb_all[:, j, 0:512], in_=psv0)
                nc.scalar.copy(out=vb_all[:, j, 512:1024], in_=psv1)

            # ctx accumulation t-major: per-pair weighted products (DVE/GpSimd
            # muls, 2:1 split) accumulate in PSUM via identity matmuls on the
            # PE (which has slack here), eliminating the DVE/GpSimd add chain.
            alt = 0
            for t in range(T):
                ctx_ps = psumc.tile([128, D], F32, tag="cps", name="cps")
                n = len(groups[t])
                for gi, (j, r0, extras) in enumerate(groups[t]):
                    pb = _bcast_free(p2_all[:, t, r0 * H : (r0 + 1) * H], DH, 0)
                    vb3 = vb_all[:, j, :].rearrange("p (dh h) -> p dh h", h=H)
                    tmp = ctmpp.tile([128, D], BF16, tag="ctmp")
                    eng = nc.gpsimd if alt % 3 == 2 else nc.vector
                    alt += 1
                    eng.tensor_mul(
                        tmp.rearrange("p (dh h) -> p dh h", h=H), vb3, pb
                    )
                    fl = dict(start=(gi == 0), stop=(gi == n - 1))
                    nc.tensor.matmul(ctx_ps[:, 0:512], ident, tmp[:, 0:512], **fl)
                    nc.tensor.matmul(
                        ctx_ps[:, 512:1024], ident, tmp[:, 512:1024], **fl
                    )
                ctx_sb = ctxsbp.tile([128, D], BF16)
                nc.scalar.copy(out=ctx_sb[:, 0:512], in_=ctx_ps[:, 0:512])
                nc.scalar.copy(out=ctx_sb[:, 512:1024], in_=ctx_ps[:, 512:1024])
                emit_out_proj(t, ctx_sb)

    return nc


_CACHE = {}


def _get_program(rel_idx, use_bq, use_bo):
    key = (rel_idx.tobytes(), use_bq, use_bo)
    if key not in _CACHE:
        nc = _build(rel_idx, use_bq, use_bo)
        nc.finalize()  # runs the bacc passes (reg alloc, wait lowering, ...)
        _CACHE[key] = nc
    return _CACHE[key]


def kernel(
    table_embs,
    rel_embs,
    rel_idx,
    Wq,
    bq,
    Wk,
    bk,
    Wv,
    bv,
    Wo,
    bo,
    w_rel,
    b_rel,
    _trace=False,
):
    table_embs = np.asarray(table_embs, dtype=np.float32)
    rel_embs = np.asarray(rel_embs, dtype=np.float32)
    rel_idx = np.asarray(rel_idx).astype(np.int64)
    Wq, Wk, Wv, Wo = (np.asarray(w, dtype=np.float32) for w in (Wq, Wk, Wv, Wo))
    bq, bk, bv, bo = (np.asarray(b, dtype=np.float32) for b in (bq, bk, bv, bo))
    w_rel = np.asarray(w_rel, dtype=np.float32)
    b_rel = np.asarray(b_rel, dtype=np.float32)

    # ---- host-side tiny prep ----
    rw = 1.0 / (1.0 + np.exp(-(rel_embs @ w_rel + b_rel[0])))  # [T, R] fp32
    bf = ml_dtypes.bfloat16
    rw_full = np.repeat(rw.astype(bf), H, axis=1)  # [T, R*H], col=r*16+h
    wq_p = np.ascontiguousarray(Wq.T[:, _PERM], dtype=bf)
    wk_p = np.ascontiguousarray(Wk.T[:, _PERM], dtype=bf)
    wv_p = np.ascontiguousarray(Wv.T[:, _PERM], dtype=bf)
    wo_p = np.ascontiguousarray(Wo.T[_PERM, :], dtype=bf)
    use_bq = bool(np.any(bq))
    bo_eff = Wo @ bv + bo
    use_bo = bool(np.any(bo_eff))
    bq_p = np.ascontiguousarray(bq[_PERM], dtype=np.float32)

    nc = _get_program(rel_idx, use_bq, use_bo)

    in_maps = []
    for c in range(NCORES):
        m = {
            "emb": np.ascontiguousarray(
                table_embs[:, c * BC : (c + 1) * BC, :], dtype=bf
            ),
            "wq": wq_p,
            "wk": wk_p,
            "wv": wv_p,
            "wo": wo_p,
            "rw": rw_full,
            "ident": np.eye(128, dtype=bf),
        }
        if use_bq:
            m["bqp"] = bq_p
        if use_bo:
            m["boe"] = bo_eff.astype(np.float32)
        in_maps.append(m)

    res = run_bass_kernel_spmd(nc, in_maps, list(range(NCORES)), trace=_trace)
    out = np.empty((T, B, D), dtype=np.float32)
    for c in range(NCORES):
        out[:, c * BC : (c + 1) * BC, :] = res.results[c]["out"].astype(np.float32)
    if _trace:
        kernel._last_results = res
    return out

